# revision 30
# baseline (speedup 1.0000x reference)
"""CLIP encoder layer on 8 trn2 NeuronCores, pure data parallel over batch.

Layout strategy (per core, batch shard of 64 sequences = 4928 tokens):
  - x arrives token-major [T, 768] fp32.
  - LayerNorm runs token-major (tokens on partitions, bn_stats/bn_aggr),
    LN scale/bias folded into the downstream projection weights host-side.
    rstd computed as exp(-0.5*ln(var+eps)) so LN shares the scalar-engine
    natural_log_exp table set with attention's EXP (no sqrt-set thrash).
  - Normalized activations are PE-transposed (bf16) to feature-major
    [768, N] for the projections (weights stationary, activations moving).
  - Attention per sequence (S=77): scoresT[k,q] = kT.T @ qT per head,
    written directly in k-major orientation so no post-softmax transpose is
    needed; 6 even heads share one PSUM bank, 6 odd heads another (row-group
    packed pairs run concurrently in the PE).  The causal mask is ADDED via
    an identity-matmul accumulation (-1e5 above diagonal) before EXP, so the
    only cross-engine dependency between scores and ctx is a single EXP per
    bank.  Softmax denominators come out of a ones-matrix matmul
    (ones77.T @ pT -> every partition holds the per-query denominator);
    normalization is fused into the ctx PSUM->SBUF evacuation multiply.
  - ctx matmuls are column-packed pairs writing [128, 77] PSUM tiles that
    are already in the ctxT feature-major layout the O-projection wants.
  - O-projection runs with swapped operands (activations stationary) so its
    output comes out token-major for the residual add (x+ob precomputed on
    the otherwise-idle GpSimd engine) and the second LayerNorm.
  - FC1 feature-major (weights stationary).  FC2 also feature-major
    (44K vs 55K PE cycles), then PE-transposed back to token-major with the
    final residual add fused into the PSUM evacuation.
  - All matmuls in bf16 (fp32 PSUM accumulation); fp32 elsewhere.
    QuickGELU via ACT Silu: x*sigmoid(1.702x) = silu(1.702x)/1.702 with the
    1/1.702 folded into fc2 weights and the 1.702 into the ACT input scale.
"""

import os
import numpy as np
import ml_dtypes

D = 768
H = 12
HD = 64
S = 77
FF = 3072
EPS = 1e-5
N_CORES = 8
B_FULL = 512
BPC = B_FULL // N_CORES          # 64 sequences per core
T_CORE = BPC * S                 # 4928 tokens per core
G_SEQ = 4                        # sequences per superblock
SB = G_SEQ * S                   # 308 tokens per superblock
MASK_NEG = -1.0e5                # additive causal mask (exp -> exact 0)


def build_program(T=T_CORE, G=G_SEQ):
    import concourse.bass as bass
    import concourse.bacc as bacc
    import concourse.mybir as mybir
    import concourse.tile as tile
    from concourse.masks import make_identity
    from contextlib import ExitStack

    f32 = mybir.dt.float32
    bf16 = mybir.dt.bfloat16
    AX = mybir.AxisListType
    OP = mybir.AluOpType
    AF = mybir.ActivationFunctionType

    SBLK = G * S
    NSB = T // SBLK
    assert NSB * SBLK == T
    NH2 = H // 2                  # 6 head pairs
    SW = NH2 * S                  # 462 score columns per bank
    # token chunks within a superblock
    chunks = []
    off = 0
    while off < SBLK:
        w = min(128, SBLK - off)
        chunks.append((off, w))
        off += w

    nc = bacc.Bacc("TRN2", target_bir_lowering=False)

    fp8 = mybir.dt.float8e4
    x_d = nc.declare_dram_parameter("x", [T, D], f32, isOutput=False)
    wq_d = nc.declare_dram_parameter("wqT", [D, D], fp8, isOutput=False)
    wk_d = nc.declare_dram_parameter("wkT", [D, D], fp8, isOutput=False)
    wv_d = nc.declare_dram_parameter("wvT", [D, D], fp8, isOutput=False)
    wo_d = nc.declare_dram_parameter("woT", [D, D], bf16, isOutput=False)
    wf1_d = nc.declare_dram_parameter("fc1T", [D, FF], fp8, isOutput=False)
    wf2_d = nc.declare_dram_parameter("fc2T", [FF, D], fp8, isOutput=False)
    qb_d = nc.declare_dram_parameter("qb", [D], f32, isOutput=False)
    kb_d = nc.declare_dram_parameter("kb", [D], f32, isOutput=False)
    vb_d = nc.declare_dram_parameter("vb", [D], f32, isOutput=False)
    ob_d = nc.declare_dram_parameter("ob", [D], f32, isOutput=False)
    f1b_d = nc.declare_dram_parameter("fc1b", [FF], f32, isOutput=False)
    f2b_d = nc.declare_dram_parameter("fc2b", [D], f32, isOutput=False)
    mask_d = nc.declare_dram_parameter("mask6", [S, SW], bf16, isOutput=False)
    out_d = nc.declare_dram_parameter("out", [T, D], f32, isOutput=True)

    with tile.TileContext(nc) as tc, ExitStack() as ctx:
        singles = ctx.enter_context(tc.tile_pool(name="singles", bufs=1))
        xpool = ctx.enter_context(tc.tile_pool(name="xpool", bufs=5))
        # x2 residual tiles must survive one extra superblock (stage F is
        # software-pipelined one SB behind attention): 3 chunks x 2 SBs.
        x2pool = ctx.enter_context(tc.tile_pool(name="x2pool", bufs=6))
        actpool = ctx.enter_context(tc.tile_pool(name="actpool", bufs=1))
        outpool = ctx.enter_context(tc.tile_pool(name="outpool", bufs=1))
        attnpool = ctx.enter_context(tc.tile_pool(name="attnpool", bufs=2))
        statpool = ctx.enter_context(tc.tile_pool(name="statpool", bufs=2))
        pspool = ctx.enter_context(tc.tile_pool(name="pspool", bufs=2, space="PSUM"))

        # ---- constants / weights ----
        wq_sb = singles.tile([128, D // 128, D], fp8)
        wk_sb = singles.tile([128, D // 128, D], fp8)
        wv_sb = singles.tile([128, D // 128, D], fp8)
        wo_sb = singles.tile([128, D // 128, D], bf16)
        wf1_sb = singles.tile([128, D // 128, FF], fp8)
        wf2_sb = singles.tile([128, FF // 128, D], fp8)
        for sb_t, dr in ((wq_sb, wq_d), (wk_sb, wk_d), (wv_sb, wv_d),
                         (wo_sb, wo_d), (wf1_sb, wf1_d), (wf2_sb, wf2_d)):
            nc.sync.dma_start(out=sb_t, in_=dr[:].rearrange("(c p) o -> p c o", p=128))

        qb_sb = singles.tile([128, D // 128], f32)
        kb_sb = singles.tile([128, D // 128], f32)
        f1b_sb = singles.tile([128, FF // 128], f32)
        f2b_sb = singles.tile([128, D // 128], f32)
        for sb_t, dr in ((qb_sb, qb_d), (kb_sb, kb_d), (f1b_sb, f1b_d),
                         (f2b_sb, f2b_d)):
            nc.sync.dma_start(out=sb_t, in_=dr[:].rearrange("(c p) -> p c", p=128))

        # free-axis biases broadcast to all 128 partitions
        ob_bc = singles.tile([128, D], f32)
        vb_bc = singles.tile([128, D], f32)
        f2b_bc = singles.tile([128, D], f32)
        for sb_t, dr in ((ob_bc, ob_d), (vb_bc, vb_d), (f2b_bc, f2b_d)):
            src = bass.AP(tensor=dr[:].tensor, offset=dr[:].offset,
                          ap=[[0, 128]] + list(dr[:].ap))
            nc.sync.dma_start(out=sb_t, in_=src)

        mask6_sb = singles.tile([S, SW], bf16)
        nc.sync.dma_start(out=mask6_sb, in_=mask_d[:])

        ident = singles.tile([128, 128], bf16)
        make_identity(nc, ident)

        ones77 = singles.tile([S, 128], bf16)
        nc.vector.memset(ones77, 1.0)

        eps_sb = singles.tile([128, 1], f32)
        nc.vector.memset(eps_sb, EPS)

        NCH = D // 128    # 6
        NFF = FF // 128   # 24

        def ln_normalize(src_tile, w, tag, bufs=2):
            """token-major [w, 768] fp32 -> normalized bf16 htok tile."""
            stats = statpool.tile([128, 3, 6], f32, tag=f"stats{tag}", name=f"stats{tag}")
            mv = statpool.tile([128, 3], f32, tag=f"mv{tag}", name=f"mv{tag}")
            xg = src_tile[:w].rearrange("p (s f) -> p s f", f=256)
            for i in range(3):
                nc.vector.bn_stats(out=stats[:w, i, :], in_=xg[:, i, :])
            nc.vector.bn_aggr(out=mv[:w, 0:2], in_=stats[:w])
            mean = mv[:w, 0:1]
            var = mv[:w, 1:2]
            lnv = mv[:w, 2:3]
            # rstd = exp(-0.5*ln(var+eps)) - stays in the natural_log_exp
            # activation-table set shared with attention's EXP.
            nc.scalar.activation(out=lnv, in_=var, func=AF.Ln,
                                 bias=eps_sb[:w], scale=1.0)
            nc.scalar.activation(out=var, in_=lnv, func=AF.Exp,
                                 bias=0.0, scale=-0.5)
            rstd = var
            htok = statpool.tile([128, D], bf16, tag=f"htok{tag}", name=f"htok{tag}",
                                 bufs=bufs)
            nc.vector.tensor_scalar(out=htok[:w], in0=src_tile[:w],
                                    scalar1=mean, scalar2=rstd,
                                    op0=OP.subtract, op1=OP.mult)
            return htok

        def seq_pieces(coff, w):
            """split token range [coff, coff+w) into per-sequence pieces of
            (src_col_in_chunk, length, dst_col) with 80-padded dst stride."""
            out_ = []
            t = coff
            while t < coff + w:
                s_ = t // S
                e = min((s_ + 1) * S, coff + w)
                out_.append((t - coff, e - t, s_ * 80 + (t - s_ * S)))
                t = e
            return out_

        def ln_transpose(htok, coff, w, dst, tag, padded=False):
            pieces = seq_pieces(coff, w) if padded else [(0, w, coff)]
            for c in range(NCH):
                ps = pspool.tile([128, 128], bf16, tag="tr", name=f"trp{tag}",
                                 bufs=2)
                nc.tensor.transpose(ps[:, :w], htok[:w, c * 128:(c + 1) * 128],
                                    ident[:w, :w])
                # evacuations alternate scalar/vector so neither in-order
                # engine queue serializes the transpose chain (ACT Copy is in
                # every activation table set -> no table loads).
                for (po_, ln_, dc_) in pieces:
                    if c % 2 == 0:
                        nc.scalar.activation(out=dst(c, dc_, ln_),
                                             in_=ps[:, po_:po_ + ln_],
                                             func=AF.Copy)
                    else:
                        nc.vector.tensor_copy(out=dst(c, dc_, ln_),
                                              in_=ps[:, po_:po_ + ln_])

        def stage_A_ln(isb):
            """load x + LN1 (vector/scalar work, emitted mid-attention of the
            previous SB so the normalized htok tiles are ready before the
            boundary); then x_tok += ob in place (idle GpSimd engine)."""
            t0 = isb * SBLK
            x_tiles = []
            htoks = []
            for (coff, w) in chunks:
                x_tok = xpool.tile([128, D], f32, tag="xtok", name="xtok")
                nc.sync.dma_start(out=x_tok[:w], in_=x_d[t0 + coff: t0 + coff + w, :])
                x_tiles.append(x_tok)
                htoks.append(ln_normalize(x_tok, w, "A", bufs=3))
                # after LN consumed raw x: fold the o-proj bias into the
                # residual in place on the idle GpSimd engine.
                nc.gpsimd.tensor_tensor(out=x_tok[:w], in0=x_tok[:w],
                                        in1=ob_bc[:w], op=OP.add)
            return htoks, x_tiles

        def stage_A_tr(htoks):
            """PE-transpose LN1 output to the feature-major fp8 layout (kept
            at the SB boundary so transpose-mode switches stay clustered)."""
            hT8 = actpool.tile([128, NCH, 320], fp8, tag="hT8", name="hT8", bufs=2)
            for ci, (coff, w) in enumerate(chunks):
                ln_transpose(htoks[ci], coff, w,
                             lambda c, o, ww: hT8[:, c, o:o + ww], "A",
                             padded=True)
            return hT8

        def stage_D_chunk(ci, ctxT, x_tiles, x2_tiles, x2f_tiles):
            coff, w = chunks[ci]
            x2 = x2pool.tile([128, D], f32, tag="x2tok", name="x2tok")
            for half in range(2):
                ps = pspool.tile([128, 384], f32, tag="big", name="pso")
                for d in range(NCH):
                    nc.tensor.matmul(ps[:w], lhsT=ctxT[d][:, coff:coff + w],
                                     rhs=wo_sb[:, d, half * 384:(half + 1) * 384],
                                     start=(d == 0), stop=(d == NCH - 1))
                sl = slice(half * 384, (half + 1) * 384)
                nc.vector.tensor_tensor(out=x2[:w, sl], in0=ps[:w],
                                        in1=x_tiles[ci][:w, sl], op=OP.add)
            x2_tiles.append(x2)

        DR = mybir.MatmulPerfMode.DoubleRow

        def stage_FC1(h2T8):
            """FC1 + QuickGELU, emitted as a dense block at the superblock
            boundary so its SILUs stay contiguous on the scalar engine (one
            silu-table load per superblock, no exp<->silu thrash)."""
            ff1_8 = actpool.tile([128, NFF, 320], fp8, tag="ff18", name="ff18",
                                 bufs=2)
            for f in range(NFF):
                ps = pspool.tile([128, SBLK], f32, tag="fc", name="psff")
                for dp in range(NCH // 2):
                    nc.tensor.matmul(ps,
                                     lhsT=wf1_sb[:, 2 * dp:2 * dp + 2,
                                                 f * 128:(f + 1) * 128],
                                     rhs=h2T8[:, 2 * dp:2 * dp + 2, :SBLK],
                                     perf_mode=DR,
                                     start=(dp == 0), stop=(dp == NCH // 2 - 1))
                # f1 = silu(1.702*(ps/16) + 1.702*b) = 1.702*quickgelu(ps+b);
                # the 1/1.702 is folded into fc2T host-side.
                nc.scalar.activation(out=ff1_8[:, f, :SBLK], in_=ps, func=AF.Silu,
                                     bias=f1b_sb[:, f:f + 1], scale=1.702 / 16)
            return ff1_8

        def stage_FC2(t0, ff1_8, x2_tiles):
            """FC2 + final residual + store, software-pipelined one SB behind.

            Pure PE/vector work (no scalar-engine activations), so the
            scheduler can interleave its dense fp8 matmuls into the next
            superblock's attention-region PE stalls -- filling dependency
            gaps and keeping the PE HAM clock-gate warm -- without
            perturbing the exp/silu activation-table sequence.
            """
            o_toks = [outpool.tile([128, D], f32, tag=f"otok{ci}", name=f"otok{ci}")
                      for ci in range(len(chunks))]
            for c in range(NCH):
                ps = pspool.tile([128, SBLK], f32, tag="fc", name="psf2")
                for fp in range(NFF // 2):
                    nc.tensor.matmul(ps,
                                     lhsT=wf2_sb[:, 2 * fp:2 * fp + 2,
                                                 c * 128:(c + 1) * 128],
                                     rhs=ff1_8[:, 2 * fp:2 * fp + 2, :SBLK],
                                     perf_mode=DR,
                                     start=(fp == 0), stop=(fp == NFF // 2 - 1))
                x3c = statpool.tile([128, SBLK], bf16, tag="x3", name="x3")
                nc.vector.tensor_scalar(out=x3c, in0=ps,
                                        scalar1=1.0 / 16, scalar2=f2b_sb[:, c:c + 1],
                                        op0=OP.mult, op1=OP.add)
                for ci, (coff, w) in enumerate(chunks):
                    tr = pspool.tile([128, 128], bf16, tag="fc", name="trf")
                    nc.tensor.transpose(tr[:w, :], x3c[:, coff:coff + w], ident)
                    nc.vector.tensor_tensor(
                        out=o_toks[ci][:w, c * 128:(c + 1) * 128],
                        in0=tr[:w, :],
                        in1=x2_tiles[ci][:w, c * 128:(c + 1) * 128], op=OP.add)
            for ci, (coff, w) in enumerate(chunks):
                nc.sync.dma_start(out=out_d[t0 + coff: t0 + coff + w, :],
                                  in_=o_toks[ci][:w])

        a0 = stage_A_ln(0)
        cur = (stage_A_tr(a0[0]), a0[1])
        next_ln = None
        fc_pending = None
        for isb in range(NSB):
            t0 = isb * SBLK
            hT8, x_tiles = cur

            # ---- stage B: q/k projections (fp8 DoubleRow, weights 16x).
            # qT/kT inherit hT8's 80-padded per-sequence column layout. ----
            qT = [actpool.tile([128, 320], bf16, tag=f"qT{c}", name=f"qT{c}")
                  for c in range(NCH)]
            kT = [actpool.tile([128, 320], bf16, tag=f"kT{c}", name=f"kT{c}")
                  for c in range(NCH)]
            for dst, w_sb, b_sb in ((qT, wq_sb, qb_sb), (kT, wk_sb, kb_sb)):
                for c in range(NCH):
                    ps = pspool.tile([128, 320], f32, tag="big", name="psqkv")
                    for dp in range(NCH // 2):
                        nc.tensor.matmul(ps,
                                         lhsT=w_sb[:, 2 * dp:2 * dp + 2,
                                                   c * 128:(c + 1) * 128],
                                         rhs=hT8[:, 2 * dp:2 * dp + 2, :],
                                         perf_mode=DR,
                                         start=(dp == 0), stop=(dp == NCH // 2 - 1))
                    nc.vector.tensor_scalar(out=dst[c], in0=ps,
                                            scalar1=1.0 / 16,
                                            scalar2=b_sb[:, c:c + 1],
                                            op0=OP.mult, op1=OP.add)

            # ---- stage C: attention per sequence ----
            ctxT = [actpool.tile([128, SBLK], bf16, tag=f"ctxT{c}", name=f"ctxT{c}",
                                 bufs=2)
                    for c in range(NCH)]
            # fp8 feature-major LN2 activations for the DoubleRow FC1
            # (padded to 320 so the k-pair free step is 16B-aligned);
            # double-buffered: stage F consumes it one superblock later.
            h2T8 = actpool.tile([128, NCH, 320], fp8, tag="h2T8", name="h2T8",
                                bufs=2)
            x2_tiles = []
            x2f_tiles = []
            h2toks = []
            next_chunk = 0
            for s in range(G):
                so = s * S        # token-contiguous column base (ctxT)
                sp = s * 80       # 80-padded column base (hT8/qT/kT)
                # v for this sequence, token-major directly (swapped operands,
                # fp8 DoubleRow; the padded hT8 base keeps offsets 16B-aligned)
                vtok = attnpool.tile([S, H, HD], bf16, tag="vtok", name="vtok")
                for half in range(2):
                    psv = pspool.tile([S, 384], f32, tag="big", name="psvtok")
                    for dp in range(NCH // 2):
                        nc.tensor.matmul(psv,
                                         lhsT=hT8[:, 2 * dp:2 * dp + 2,
                                                  sp:sp + S],
                                         rhs=wv_sb[:, 2 * dp:2 * dp + 2,
                                                   half * 384:(half + 1) * 384],
                                         perf_mode=DR,
                                         start=(dp == 0), stop=(dp == NCH // 2 - 1))
                    nc.vector.scalar_tensor_tensor(
                        out=vtok[:, half * 6:(half + 1) * 6, :],
                        in0=psv, scalar=1.0 / 16,
                        in1=vb_bc[:S, half * 384:(half + 1) * 384],
                        op0=OP.mult, op1=OP.add)
                # scoresT[k, q] per head; even heads -> bank A, odd -> bank B.
                # Row-group packing: even heads live on partitions 0:64 of
                # their qT/kT chunk, odd heads on 64:128 -> pairs overlap.
                psc = [pspool.tile([S, SW], f32, tag="sc", name=f"psc{a}")
                       for a in range(2)]
                for j in range(NH2):
                    for a in range(2):
                        h = 2 * j + a
                        c, po = h // 2, 64 * (h % 2)
                        nc.tensor.matmul(psc[a][:, j * S:(j + 1) * S],
                                         lhsT=kT[c][po:po + 64, sp:sp + S],
                                         rhs=qT[c][po:po + 64, sp:sp + S],
                                         start=(j == 0), stop=False,
                                         skip_group_check=True)
                # additive causal mask via identity-matmul accumulation
                pT = attnpool.tile([S, 2, SW], bf16, tag="pT", name="pT")
                for a in range(2):
                    nc.tensor.matmul(psc[a], lhsT=ident[:S, :S], rhs=mask6_sb,
                                     start=False, stop=True, skip_group_check=True)
                    nc.scalar.activation(out=pT[:, a, :], in_=psc[a], func=AF.Exp)
                # denominators broadcast across partitions: ones.T @ pT, the
                # two banks column-packed into one PSUM tile (concurrent).
                dben = pspool.tile([128, SW], f32, tag="sc", name="dben")
                rp = attnpool.tile([128, SW], bf16, tag="rp", name="rp")
                lnd = attnpool.tile([128, SW], f32, tag="lnd", name="lnd")
                for a in range(2):
                    nc.tensor.matmul(dben[64 * a:64 * a + 64, :],
                                     lhsT=ones77[:, 64 * a:64 * a + 64],
                                     rhs=pT[:, a, :],
                                     start=True, stop=True,
                                     skip_group_check=True)
                # reciprocal as exp(-ln(x)) on the scalar engine: same table
                # set as the attention EXP, and off the busy vector engine
                # (nc.vector.reciprocal is ~6.5ns/elem - 3us per call here).
                # One full-width op per step: ACT cost scales with free dim
                # only, so [128,462] costs the same as [64,462].
                nc.scalar.activation(out=lnd, in_=dben, func=AF.Ln)
                nc.scalar.activation(out=rp, in_=lnd, func=AF.Exp, scale=-1.0)
                # ctx per head pair, column-packed into [128, 77] PSUM already
                # in ctxT layout; normalization fused into the evacuation.
                for j in range(NH2):
                    ctxp = pspool.tile([128, S], f32, tag="tr", name="ctxp",
                                       bufs=2)
                    for a in range(2):
                        h = 2 * j + a
                        nc.tensor.matmul(ctxp[64 * a:64 * a + 64, :],
                                         lhsT=vtok[:, h, :],
                                         rhs=pT[:, a, j * S:(j + 1) * S],
                                         start=True, stop=True,
                                         skip_group_check=True)
                    nc.vector.tensor_tensor(out=ctxT[j][:, so:so + S], in0=ctxp,
                                            in1=rp[:, j * S:(j + 1) * S],
                                            op=OP.mult)
                # prefetch next SB's x-loads + LN1 mid-attention (vector/
                # scalar work only; the PE transposes stay at the boundary)
                if s == 1 and isb + 1 < NSB:
                    next_ln = stage_A_ln(isb + 1)

                # emit O-proj + residual + LN2 for chunks fully covered
                done_tokens = (s + 1) * S
                while (next_chunk < len(chunks)
                       and chunks[next_chunk][0] + chunks[next_chunk][1]
                       <= done_tokens):
                    ci = next_chunk
                    stage_D_chunk(ci, ctxT, x_tiles, x2_tiles, x2f_tiles)
                    coff, w = chunks[ci]
                    h2toks.append(ln_normalize(x2_tiles[ci], w, "E", bufs=3))
                    next_chunk += 1

            # ---- next superblock's LN1 transposes (LN itself already ran
            # mid-attention); clustered here so the PE's transpose-mode
            # switches stay batched ----
            if isb + 1 < NSB:
                cur = (stage_A_tr(next_ln[0]), next_ln[1])

            # E transposes (emitted after C so the in-order PE isn't blocked
            # mid-attention waiting on the LN chains; batched here so the
            # PE's transpose-mode switches stay clustered)
            for ci, (coff, w) in enumerate(chunks):
                ln_transpose(h2toks[ci], coff, w,
                             lambda c, o, ww: h2T8[:, c, o:o + ww], "E")

            # ---- the WHOLE MLP of the previous superblock (FC1 then FC2)
            # emitted here at low priority: FC1(i-1) is ready from the start
            # of this SB, so the filler stream covers this SB's attention
            # stalls, and FC2(i-1) remnants are still in flight to cover
            # the boundary transpose chains.  (FC1's silu<->exp table loads
            # cost ~1.3us each on the scalar engine, but ACT has slack and
            # the PE overlap is worth more.) ----
            if fc_pending is not None:
                pt0, ph2T8, px2 = fc_pending
                ff1_8 = stage_FC1(ph2T8)
                stage_FC2(pt0, ff1_8, px2)
            fc_pending = (t0, h2T8, x2_tiles)
        pt0, ph2T8, px2 = fc_pending
        ff1_8 = stage_FC1(ph2T8)
        stage_FC2(pt0, ff1_8, px2)

    # Restrict the activation-table-set chooser to the two sets that cover
    # everything this kernel uses (ln+exp share one set; silu the other).
    # Entries keep their original indices (act_func_set_id is positional);
    # unwanted sets are just emptied so the chooser can never pick them.
    from concourse.hw_specs import get_activation_tables
    import bass_rust as _bass_rust
    _tables = list(get_activation_tables(nc.m.arch).items())
    _keep = {"natural_log_exp_and_others", "silu_and_others"}
    _tables = [(n, (f if n in _keep else set())) for (n, f) in _tables]

    def _patched_insert_act_table_loads():
        _bass_rust.insert_act_table_loads(nc, _tables)

    nc.insert_act_table_loads = _patched_insert_act_table_loads
    nc.compile()
    return nc


def prep_shared(inputs):
    """Fold LN affine params / scale constants into weights -> shared in_map entries."""
    bf = ml_dtypes.bfloat16
    f32 = np.float32
    g = {k: np.asarray(v, dtype=np.float32) for k, v in inputs.items() if k != "x"}

    # projection/MLP weights in fp8 e4m3, scaled 16x so the smallest weights
    # stay out of the subnormal floor; the 1/16 is folded into the PSUM
    # evacuations (tensor_scalar mult / SILU input scale).
    e4 = ml_dtypes.float8_e4m3
    wqT = (g["ln1_w"][:, None] * g["qw"].T * 0.125 * 16.0).astype(e4)
    wkT = (g["ln1_w"][:, None] * g["kw"].T * 16.0).astype(e4)
    wvT = (g["ln1_w"][:, None] * g["vw"].T * 16.0).astype(e4)
    woT = np.ascontiguousarray(g["ow"].T).astype(bf)
    fc1T = (g["ln2_w"][:, None] * g["fc1_w"].T * 16.0).astype(e4)
    fc2T = (g["fc2_w"].T / 1.702 * 16.0).astype(e4)

    qb = ((g["ln1_b"] @ g["qw"].T + g["qb"]) * 0.125).astype(f32)
    kb = (g["ln1_b"] @ g["kw"].T + g["kb"]).astype(f32)
    vb = (g["ln1_b"] @ g["vw"].T + g["vb"]).astype(f32)
    ob = g["ob"].astype(f32)
    fc1b = ((g["ln2_b"] @ g["fc1_w"].T + g["fc1_b"]) * 1.702).astype(f32)
    fc2b = g["fc2_b"].astype(f32)

    # additive causal mask in scoresT[k, q] orientation (k > q masked),
    # tiled 6x along q for the per-bank [77, 462] accumulation matmul.
    m1 = np.where(np.arange(S)[:, None] > np.arange(S)[None, :], MASK_NEG, 0.0)
    mask6 = np.tile(m1.astype(np.float32), (1, H // 2)).astype(bf)

    return dict(wqT=wqT, wkT=wkT, wvT=wvT, woT=woT, fc1T=fc1T, fc2T=fc2T,
                qb=qb, kb=kb, vb=vb, ob=ob, fc1b=fc1b, fc2b=fc2b, mask6=mask6)


def prep_host_inputs(inputs):
    shared = prep_shared(inputs)
    x = np.asarray(inputs["x"], dtype=np.float32)
    in_maps = []
    for c in range(N_CORES):
        xc = np.ascontiguousarray(
            x[c * BPC:(c + 1) * BPC].reshape(T_CORE, D).astype(np.float32))
        in_maps.append(dict(shared, x=xc))
    return in_maps


_CACHED_NC = None


def _get_nc():
    global _CACHED_NC
    if _CACHED_NC is None:
        _CACHED_NC = build_program()
    return _CACHED_NC


def run(inputs, trace=False):
    from concourse.bass_utils import run_bass_kernel_spmd
    nc = _get_nc()
    in_maps = prep_host_inputs(inputs)
    res = run_bass_kernel_spmd(nc, in_maps, list(range(N_CORES)), trace=trace)
    outs = [np.asarray(res.results[c]["out"], dtype=np.float32).reshape(BPC, S, D)
            for c in range(N_CORES)]
    full = np.concatenate(outs, axis=0)
    return full, res


def kernel(**inputs):
    full, _ = run(inputs, trace=False)
    return full



# revision 31
# speedup vs baseline: 1.0489x; 1.0489x over previous
"""CLIP encoder layer on 8 trn2 NeuronCores, pure data parallel over batch.

Layout strategy (per core, batch shard of 64 sequences = 4928 tokens):
  - x arrives token-major [T, 768] fp32.
  - LayerNorm runs token-major (tokens on partitions, bn_stats/bn_aggr),
    LN scale/bias folded into the downstream projection weights host-side.
    rstd computed as exp(-0.5*ln(var+eps)) so LN shares the scalar-engine
    natural_log_exp table set with attention's EXP (no sqrt-set thrash).
  - Normalized activations are PE-transposed (bf16) to feature-major
    [768, N] for the projections (weights stationary, activations moving).
  - Attention per sequence (S=77): scoresT[k,q] = kT.T @ qT per head,
    written directly in k-major orientation so no post-softmax transpose is
    needed; 6 even heads share one PSUM bank, 6 odd heads another (row-group
    packed pairs run concurrently in the PE).  The causal mask is ADDED via
    an identity-matmul accumulation (-1e5 above diagonal) before EXP, so the
    only cross-engine dependency between scores and ctx is a single EXP per
    bank.  Softmax denominators come out of a ones-matrix matmul
    (ones77.T @ pT -> every partition holds the per-query denominator);
    normalization is fused into the ctx PSUM->SBUF evacuation multiply.
  - ctx matmuls are column-packed pairs writing [128, 77] PSUM tiles that
    are already in the ctxT feature-major layout the O-projection wants.
  - O-projection runs with swapped operands (activations stationary) so its
    output comes out token-major for the residual add (x+ob precomputed on
    the otherwise-idle GpSimd engine) and the second LayerNorm.
  - FC1 feature-major (weights stationary).  FC2 also feature-major
    (44K vs 55K PE cycles), then PE-transposed back to token-major with the
    final residual add fused into the PSUM evacuation.
  - All matmuls in bf16 (fp32 PSUM accumulation); fp32 elsewhere.
    QuickGELU via ACT Silu: x*sigmoid(1.702x) = silu(1.702x)/1.702 with the
    1/1.702 folded into fc2 weights and the 1.702 into the ACT input scale.

Scheduling (the big lever on this kernel -- the attention phase alone
leaves the PE idle/HAM-throttled):
  - The whole MLP (FC1+FC2) is software-pipelined ONE superblock behind
    attention and emitted at the lowest priority, so its dense fp8 matmul
    stream statically fills the attention-phase dependency stalls and the
    boundary transpose chains, keeping the PE's HAM clock-gate warm.
    (Its SILUs do interleave with attention's EXP/LN ops, costing ~11
    activation-table loads per SB on the scalar engine, but ACT has slack
    and the PE overlap is worth far more.)
  - The next superblock's x-loads + LN1 run mid-attention (vector/scalar
    work); only the PE transposes stay at the boundary, batched so
    transpose-mode switches stay clustered.
  - Transpose PSUM evacuations alternate scalar/vector (ACT Copy is in
    every table set); the softmax reciprocal's Ln/Exp run as single
    full-width [128,462] ops (ACT cost scales with free size only).
  - PSUM banks: tr+ctx share one 2-buf tag (their phases are temporally
    disjoint), qkv/v/o-proj share "big", scores "sc", FC1/FC2 "fc".
"""

import os
import numpy as np
import ml_dtypes

D = 768
H = 12
HD = 64
S = 77
FF = 3072
EPS = 1e-5
N_CORES = 8
B_FULL = 512
BPC = B_FULL // N_CORES          # 64 sequences per core
T_CORE = BPC * S                 # 4928 tokens per core
G_SEQ = 4                        # sequences per superblock
SB = G_SEQ * S                   # 308 tokens per superblock
MASK_NEG = -1.0e5                # additive causal mask (exp -> exact 0)


def build_program(T=T_CORE, G=G_SEQ):
    import concourse.bass as bass
    import concourse.bacc as bacc
    import concourse.mybir as mybir
    import concourse.tile as tile
    from concourse.masks import make_identity
    from contextlib import ExitStack

    f32 = mybir.dt.float32
    bf16 = mybir.dt.bfloat16
    AX = mybir.AxisListType
    OP = mybir.AluOpType
    AF = mybir.ActivationFunctionType

    SBLK = G * S
    NSB = T // SBLK
    assert NSB * SBLK == T
    NH2 = H // 2                  # 6 head pairs
    SW = NH2 * S                  # 462 score columns per bank
    # token chunks within a superblock
    chunks = []
    off = 0
    while off < SBLK:
        w = min(128, SBLK - off)
        chunks.append((off, w))
        off += w

    nc = bacc.Bacc("TRN2", target_bir_lowering=False)

    fp8 = mybir.dt.float8e4
    x_d = nc.declare_dram_parameter("x", [T, D], f32, isOutput=False)
    wq_d = nc.declare_dram_parameter("wqT", [D, D], fp8, isOutput=False)
    wk_d = nc.declare_dram_parameter("wkT", [D, D], fp8, isOutput=False)
    wv_d = nc.declare_dram_parameter("wvT", [D, D], fp8, isOutput=False)
    wo_d = nc.declare_dram_parameter("woT", [D, D], bf16, isOutput=False)
    wf1_d = nc.declare_dram_parameter("fc1T", [D, FF], fp8, isOutput=False)
    wf2_d = nc.declare_dram_parameter("fc2T", [FF, D], fp8, isOutput=False)
    qb_d = nc.declare_dram_parameter("qb", [D], f32, isOutput=False)
    kb_d = nc.declare_dram_parameter("kb", [D], f32, isOutput=False)
    vb_d = nc.declare_dram_parameter("vb", [D], f32, isOutput=False)
    ob_d = nc.declare_dram_parameter("ob", [D], f32, isOutput=False)
    f1b_d = nc.declare_dram_parameter("fc1b", [FF], f32, isOutput=False)
    f2b_d = nc.declare_dram_parameter("fc2b", [D], f32, isOutput=False)
    mask_d = nc.declare_dram_parameter("mask6", [S, SW], bf16, isOutput=False)
    out_d = nc.declare_dram_parameter("out", [T, D], f32, isOutput=True)

    with tile.TileContext(nc) as tc, ExitStack() as ctx:
        singles = ctx.enter_context(tc.tile_pool(name="singles", bufs=1))
        xpool = ctx.enter_context(tc.tile_pool(name="xpool", bufs=5))
        # x2 residual tiles must survive one extra superblock (stage F is
        # software-pipelined one SB behind attention): 3 chunks x 2 SBs.
        x2pool = ctx.enter_context(tc.tile_pool(name="x2pool", bufs=6))
        actpool = ctx.enter_context(tc.tile_pool(name="actpool", bufs=1))
        outpool = ctx.enter_context(tc.tile_pool(name="outpool", bufs=1))
        attnpool = ctx.enter_context(tc.tile_pool(name="attnpool", bufs=2))
        statpool = ctx.enter_context(tc.tile_pool(name="statpool", bufs=2))
        pspool = ctx.enter_context(tc.tile_pool(name="pspool", bufs=2, space="PSUM"))

        # ---- constants / weights ----
        wq_sb = singles.tile([128, D // 128, D], fp8)
        wk_sb = singles.tile([128, D // 128, D], fp8)
        wv_sb = singles.tile([128, D // 128, D], fp8)
        wo_sb = singles.tile([128, D // 128, D], bf16)
        wf1_sb = singles.tile([128, D // 128, FF], fp8)
        wf2_sb = singles.tile([128, FF // 128, D], fp8)
        for sb_t, dr in ((wq_sb, wq_d), (wk_sb, wk_d), (wv_sb, wv_d),
                         (wo_sb, wo_d), (wf1_sb, wf1_d), (wf2_sb, wf2_d)):
            nc.sync.dma_start(out=sb_t, in_=dr[:].rearrange("(c p) o -> p c o", p=128))

        qb_sb = singles.tile([128, D // 128], f32)
        kb_sb = singles.tile([128, D // 128], f32)
        f1b_sb = singles.tile([128, FF // 128], f32)
        f2b_sb = singles.tile([128, D // 128], f32)
        for sb_t, dr in ((qb_sb, qb_d), (kb_sb, kb_d), (f1b_sb, f1b_d),
                         (f2b_sb, f2b_d)):
            nc.sync.dma_start(out=sb_t, in_=dr[:].rearrange("(c p) -> p c", p=128))

        # free-axis biases broadcast to all 128 partitions
        ob_bc = singles.tile([128, D], f32)
        vb_bc = singles.tile([128, D], f32)
        f2b_bc = singles.tile([128, D], f32)
        for sb_t, dr in ((ob_bc, ob_d), (vb_bc, vb_d), (f2b_bc, f2b_d)):
            src = bass.AP(tensor=dr[:].tensor, offset=dr[:].offset,
                          ap=[[0, 128]] + list(dr[:].ap))
            nc.sync.dma_start(out=sb_t, in_=src)

        mask6_sb = singles.tile([S, SW], bf16)
        nc.sync.dma_start(out=mask6_sb, in_=mask_d[:])

        ident = singles.tile([128, 128], bf16)
        make_identity(nc, ident)

        ones77 = singles.tile([S, 128], bf16)
        nc.vector.memset(ones77, 1.0)

        eps_sb = singles.tile([128, 1], f32)
        nc.vector.memset(eps_sb, EPS)

        NCH = D // 128    # 6
        NFF = FF // 128   # 24

        def ln_normalize(src_tile, w, tag, bufs=2):
            """token-major [w, 768] fp32 -> normalized bf16 htok tile."""
            stats = statpool.tile([128, 3, 6], f32, tag=f"stats{tag}", name=f"stats{tag}")
            mv = statpool.tile([128, 3], f32, tag=f"mv{tag}", name=f"mv{tag}")
            xg = src_tile[:w].rearrange("p (s f) -> p s f", f=256)
            for i in range(3):
                nc.vector.bn_stats(out=stats[:w, i, :], in_=xg[:, i, :])
            nc.vector.bn_aggr(out=mv[:w, 0:2], in_=stats[:w])
            mean = mv[:w, 0:1]
            var = mv[:w, 1:2]
            lnv = mv[:w, 2:3]
            # rstd = exp(-0.5*ln(var+eps)) - stays in the natural_log_exp
            # activation-table set shared with attention's EXP.
            nc.scalar.activation(out=lnv, in_=var, func=AF.Ln,
                                 bias=eps_sb[:w], scale=1.0)
            nc.scalar.activation(out=var, in_=lnv, func=AF.Exp,
                                 bias=0.0, scale=-0.5)
            rstd = var
            htok = statpool.tile([128, D], bf16, tag=f"htok{tag}", name=f"htok{tag}",
                                 bufs=bufs)
            nc.vector.tensor_scalar(out=htok[:w], in0=src_tile[:w],
                                    scalar1=mean, scalar2=rstd,
                                    op0=OP.subtract, op1=OP.mult)
            return htok

        def seq_pieces(coff, w):
            """split token range [coff, coff+w) into per-sequence pieces of
            (src_col_in_chunk, length, dst_col) with 80-padded dst stride."""
            out_ = []
            t = coff
            while t < coff + w:
                s_ = t // S
                e = min((s_ + 1) * S, coff + w)
                out_.append((t - coff, e - t, s_ * 80 + (t - s_ * S)))
                t = e
            return out_

        def ln_transpose(htok, coff, w, dst, tag, padded=False):
            pieces = seq_pieces(coff, w) if padded else [(0, w, coff)]
            for c in range(NCH):
                ps = pspool.tile([128, 128], bf16, tag="tr", name=f"trp{tag}",
                                 bufs=2)
                nc.tensor.transpose(ps[:, :w], htok[:w, c * 128:(c + 1) * 128],
                                    ident[:w, :w])
                # evacuations alternate scalar/vector so neither in-order
                # engine queue serializes the transpose chain (ACT Copy is in
                # every activation table set -> no table loads).
                for (po_, ln_, dc_) in pieces:
                    if c % 2 == 0:
                        nc.scalar.activation(out=dst(c, dc_, ln_),
                                             in_=ps[:, po_:po_ + ln_],
                                             func=AF.Copy)
                    else:
                        nc.vector.tensor_copy(out=dst(c, dc_, ln_),
                                              in_=ps[:, po_:po_ + ln_])

        def stage_A_ln(isb):
            """load x + LN1 (vector/scalar work, emitted mid-attention of the
            previous SB so the normalized htok tiles are ready before the
            boundary); then x_tok += ob in place (idle GpSimd engine)."""
            t0 = isb * SBLK
            x_tiles = []
            htoks = []
            for (coff, w) in chunks:
                x_tok = xpool.tile([128, D], f32, tag="xtok", name="xtok")
                nc.sync.dma_start(out=x_tok[:w], in_=x_d[t0 + coff: t0 + coff + w, :])
                x_tiles.append(x_tok)
                htoks.append(ln_normalize(x_tok, w, "A", bufs=3))
                # after LN consumed raw x: fold the o-proj bias into the
                # residual in place on the idle GpSimd engine.
                nc.gpsimd.tensor_tensor(out=x_tok[:w], in0=x_tok[:w],
                                        in1=ob_bc[:w], op=OP.add)
            return htoks, x_tiles

        def stage_A_tr(htoks):
            """PE-transpose LN1 output to the feature-major fp8 layout (kept
            at the SB boundary so transpose-mode switches stay clustered)."""
            hT8 = actpool.tile([128, NCH, 320], fp8, tag="hT8", name="hT8", bufs=2)
            for ci, (coff, w) in enumerate(chunks):
                ln_transpose(htoks[ci], coff, w,
                             lambda c, o, ww: hT8[:, c, o:o + ww], "A",
                             padded=True)
            return hT8

        def stage_D_chunk(ci, ctxT, x_tiles, x2_tiles, x2f_tiles):
            coff, w = chunks[ci]
            x2 = x2pool.tile([128, D], f32, tag="x2tok", name="x2tok")
            for half in range(2):
                ps = pspool.tile([128, 384], f32, tag="big", name="pso")
                for d in range(NCH):
                    nc.tensor.matmul(ps[:w], lhsT=ctxT[d][:, coff:coff + w],
                                     rhs=wo_sb[:, d, half * 384:(half + 1) * 384],
                                     start=(d == 0), stop=(d == NCH - 1))
                sl = slice(half * 384, (half + 1) * 384)
                nc.vector.tensor_tensor(out=x2[:w, sl], in0=ps[:w],
                                        in1=x_tiles[ci][:w, sl], op=OP.add)
            x2_tiles.append(x2)

        DR = mybir.MatmulPerfMode.DoubleRow

        def stage_FC1(h2T8):
            """FC1 + QuickGELU, emitted as a dense block at the superblock
            boundary so its SILUs stay contiguous on the scalar engine (one
            silu-table load per superblock, no exp<->silu thrash)."""
            ff1_8 = actpool.tile([128, NFF, 320], fp8, tag="ff18", name="ff18",
                                 bufs=2)
            for f in range(NFF):
                ps = pspool.tile([128, SBLK], f32, tag="fc", name="psff")
                for dp in range(NCH // 2):
                    nc.tensor.matmul(ps,
                                     lhsT=wf1_sb[:, 2 * dp:2 * dp + 2,
                                                 f * 128:(f + 1) * 128],
                                     rhs=h2T8[:, 2 * dp:2 * dp + 2, :SBLK],
                                     perf_mode=DR,
                                     start=(dp == 0), stop=(dp == NCH // 2 - 1))
                # f1 = silu(1.702*(ps/16) + 1.702*b) = 1.702*quickgelu(ps+b);
                # the 1/1.702 is folded into fc2T host-side.
                nc.scalar.activation(out=ff1_8[:, f, :SBLK], in_=ps, func=AF.Silu,
                                     bias=f1b_sb[:, f:f + 1], scale=1.702 / 16)
            return ff1_8

        def stage_FC2(t0, ff1_8, x2_tiles):
            """FC2 + final residual + store, software-pipelined one SB behind.

            Pure PE/vector work (no scalar-engine activations), so the
            scheduler can interleave its dense fp8 matmuls into the next
            superblock's attention-region PE stalls -- filling dependency
            gaps and keeping the PE HAM clock-gate warm -- without
            perturbing the exp/silu activation-table sequence.
            """
            o_toks = [outpool.tile([128, D], f32, tag=f"otok{ci}", name=f"otok{ci}")
                      for ci in range(len(chunks))]
            for c in range(NCH):
                ps = pspool.tile([128, SBLK], f32, tag="fc", name="psf2")
                for fp in range(NFF // 2):
                    nc.tensor.matmul(ps,
                                     lhsT=wf2_sb[:, 2 * fp:2 * fp + 2,
                                                 c * 128:(c + 1) * 128],
                                     rhs=ff1_8[:, 2 * fp:2 * fp + 2, :SBLK],
                                     perf_mode=DR,
                                     start=(fp == 0), stop=(fp == NFF // 2 - 1))
                x3c = statpool.tile([128, SBLK], bf16, tag="x3", name="x3")
                nc.vector.tensor_scalar(out=x3c, in0=ps,
                                        scalar1=1.0 / 16, scalar2=f2b_sb[:, c:c + 1],
                                        op0=OP.mult, op1=OP.add)
                for ci, (coff, w) in enumerate(chunks):
                    tr = pspool.tile([128, 128], bf16, tag="fc", name="trf")
                    nc.tensor.transpose(tr[:w, :], x3c[:, coff:coff + w], ident)
                    nc.vector.tensor_tensor(
                        out=o_toks[ci][:w, c * 128:(c + 1) * 128],
                        in0=tr[:w, :],
                        in1=x2_tiles[ci][:w, c * 128:(c + 1) * 128], op=OP.add)
            for ci, (coff, w) in enumerate(chunks):
                nc.sync.dma_start(out=out_d[t0 + coff: t0 + coff + w, :],
                                  in_=o_toks[ci][:w])

        a0 = stage_A_ln(0)
        cur = (stage_A_tr(a0[0]), a0[1])
        next_ln = None
        fc_pending = None
        for isb in range(NSB):
            t0 = isb * SBLK
            hT8, x_tiles = cur

            # ---- stage B: q/k projections (fp8 DoubleRow, weights 16x).
            # qT/kT inherit hT8's 80-padded per-sequence column layout. ----
            qT = [actpool.tile([128, 320], bf16, tag=f"qT{c}", name=f"qT{c}")
                  for c in range(NCH)]
            kT = [actpool.tile([128, 320], bf16, tag=f"kT{c}", name=f"kT{c}")
                  for c in range(NCH)]
            for dst, w_sb, b_sb in ((qT, wq_sb, qb_sb), (kT, wk_sb, kb_sb)):
                for c in range(NCH):
                    ps = pspool.tile([128, 320], f32, tag="big", name="psqkv")
                    for dp in range(NCH // 2):
                        nc.tensor.matmul(ps,
                                         lhsT=w_sb[:, 2 * dp:2 * dp + 2,
                                                   c * 128:(c + 1) * 128],
                                         rhs=hT8[:, 2 * dp:2 * dp + 2, :],
                                         perf_mode=DR,
                                         start=(dp == 0), stop=(dp == NCH // 2 - 1))
                    nc.vector.tensor_scalar(out=dst[c], in0=ps,
                                            scalar1=1.0 / 16,
                                            scalar2=b_sb[:, c:c + 1],
                                            op0=OP.mult, op1=OP.add)

            # ---- stage C: attention per sequence ----
            ctxT = [actpool.tile([128, SBLK], bf16, tag=f"ctxT{c}", name=f"ctxT{c}",
                                 bufs=2)
                    for c in range(NCH)]
            # fp8 feature-major LN2 activations for the DoubleRow FC1
            # (padded to 320 so the k-pair free step is 16B-aligned);
            # double-buffered: stage F consumes it one superblock later.
            h2T8 = actpool.tile([128, NCH, 320], fp8, tag="h2T8", name="h2T8",
                                bufs=2)
            x2_tiles = []
            x2f_tiles = []
            h2toks = []
            next_chunk = 0
            for s in range(G):
                so = s * S        # token-contiguous column base (ctxT)
                sp = s * 80       # 80-padded column base (hT8/qT/kT)
                # v for this sequence, token-major directly (swapped operands,
                # fp8 DoubleRow; the padded hT8 base keeps offsets 16B-aligned)
                vtok = attnpool.tile([S, H, HD], bf16, tag="vtok", name="vtok")
                for half in range(2):
                    psv = pspool.tile([S, 384], f32, tag="big", name="psvtok")
                    for dp in range(NCH // 2):
                        nc.tensor.matmul(psv,
                                         lhsT=hT8[:, 2 * dp:2 * dp + 2,
                                                  sp:sp + S],
                                         rhs=wv_sb[:, 2 * dp:2 * dp + 2,
                                                   half * 384:(half + 1) * 384],
                                         perf_mode=DR,
                                         start=(dp == 0), stop=(dp == NCH // 2 - 1))
                    nc.vector.scalar_tensor_tensor(
                        out=vtok[:, half * 6:(half + 1) * 6, :],
                        in0=psv, scalar=1.0 / 16,
                        in1=vb_bc[:S, half * 384:(half + 1) * 384],
                        op0=OP.mult, op1=OP.add)
                # scoresT[k, q] per head; even heads -> bank A, odd -> bank B.
                # Row-group packing: even heads live on partitions 0:64 of
                # their qT/kT chunk, odd heads on 64:128 -> pairs overlap.
                psc = [pspool.tile([S, SW], f32, tag="sc", name=f"psc{a}")
                       for a in range(2)]
                for j in range(NH2):
                    for a in range(2):
                        h = 2 * j + a
                        c, po = h // 2, 64 * (h % 2)
                        nc.tensor.matmul(psc[a][:, j * S:(j + 1) * S],
                                         lhsT=kT[c][po:po + 64, sp:sp + S],
                                         rhs=qT[c][po:po + 64, sp:sp + S],
                                         start=(j == 0), stop=False,
                                         skip_group_check=True)
                # additive causal mask via identity-matmul accumulation
                pT = attnpool.tile([S, 2, SW], bf16, tag="pT", name="pT")
                for a in range(2):
                    nc.tensor.matmul(psc[a], lhsT=ident[:S, :S], rhs=mask6_sb,
                                     start=False, stop=True, skip_group_check=True)
                    nc.scalar.activation(out=pT[:, a, :], in_=psc[a], func=AF.Exp)
                # denominators broadcast across partitions: ones.T @ pT, the
                # two banks column-packed into one PSUM tile (concurrent).
                dben = pspool.tile([128, SW], f32, tag="sc", name="dben")
                rp = attnpool.tile([128, SW], bf16, tag="rp", name="rp")
                lnd = attnpool.tile([128, SW], f32, tag="lnd", name="lnd")
                for a in range(2):
                    nc.tensor.matmul(dben[64 * a:64 * a + 64, :],
                                     lhsT=ones77[:, 64 * a:64 * a + 64],
                                     rhs=pT[:, a, :],
                                     start=True, stop=True,
                                     skip_group_check=True)
                # reciprocal as exp(-ln(x)) on the scalar engine: same table
                # set as the attention EXP, and off the busy vector engine
                # (nc.vector.reciprocal is ~6.5ns/elem - 3us per call here).
                # One full-width op per step: ACT cost scales with free dim
                # only, so [128,462] costs the same as [64,462].
                nc.scalar.activation(out=lnd, in_=dben, func=AF.Ln)
                nc.scalar.activation(out=rp, in_=lnd, func=AF.Exp, scale=-1.0)
                # ctx per head pair, column-packed into [128, 77] PSUM already
                # in ctxT layout; normalization fused into the evacuation.
                for j in range(NH2):
                    ctxp = pspool.tile([128, S], f32, tag="tr", name="ctxp",
                                       bufs=2)
                    for a in range(2):
                        h = 2 * j + a
                        nc.tensor.matmul(ctxp[64 * a:64 * a + 64, :],
                                         lhsT=vtok[:, h, :],
                                         rhs=pT[:, a, j * S:(j + 1) * S],
                                         start=True, stop=True,
                                         skip_group_check=True)
                    nc.vector.tensor_tensor(out=ctxT[j][:, so:so + S], in0=ctxp,
                                            in1=rp[:, j * S:(j + 1) * S],
                                            op=OP.mult)
                # prefetch next SB's x-loads + LN1 mid-attention (vector/
                # scalar work only; the PE transposes stay at the boundary)
                if s == 1 and isb + 1 < NSB:
                    next_ln = stage_A_ln(isb + 1)

                # emit O-proj + residual + LN2 for chunks fully covered
                done_tokens = (s + 1) * S
                while (next_chunk < len(chunks)
                       and chunks[next_chunk][0] + chunks[next_chunk][1]
                       <= done_tokens):
                    ci = next_chunk
                    stage_D_chunk(ci, ctxT, x_tiles, x2_tiles, x2f_tiles)
                    coff, w = chunks[ci]
                    h2toks.append(ln_normalize(x2_tiles[ci], w, "E", bufs=3))
                    next_chunk += 1

            # ---- next superblock's LN1 transposes (LN itself already ran
            # mid-attention); clustered here so the PE's transpose-mode
            # switches stay batched ----
            if isb + 1 < NSB:
                cur = (stage_A_tr(next_ln[0]), next_ln[1])

            # E transposes (emitted after C so the in-order PE isn't blocked
            # mid-attention waiting on the LN chains; batched here so the
            # PE's transpose-mode switches stay clustered)
            for ci, (coff, w) in enumerate(chunks):
                ln_transpose(h2toks[ci], coff, w,
                             lambda c, o, ww: h2T8[:, c, o:o + ww], "E")

            # ---- the WHOLE MLP of the previous superblock (FC1 then FC2)
            # emitted here at low priority: FC1(i-1) is ready from the start
            # of this SB, so the filler stream covers this SB's attention
            # stalls, and FC2(i-1) remnants are still in flight to cover
            # the boundary transpose chains.  (FC1's silu<->exp table loads
            # cost ~1.3us each on the scalar engine, but ACT has slack and
            # the PE overlap is worth more.) ----
            if fc_pending is not None:
                pt0, ph2T8, px2 = fc_pending
                ff1_8 = stage_FC1(ph2T8)
                stage_FC2(pt0, ff1_8, px2)
            fc_pending = (t0, h2T8, x2_tiles)
        pt0, ph2T8, px2 = fc_pending
        ff1_8 = stage_FC1(ph2T8)
        stage_FC2(pt0, ff1_8, px2)

    # Restrict the activation-table-set chooser to the two sets that cover
    # everything this kernel uses (ln+exp share one set; silu the other).
    # Entries keep their original indices (act_func_set_id is positional);
    # unwanted sets are just emptied so the chooser can never pick them.
    from concourse.hw_specs import get_activation_tables
    import bass_rust as _bass_rust
    _tables = list(get_activation_tables(nc.m.arch).items())
    _keep = {"natural_log_exp_and_others", "silu_and_others"}
    _tables = [(n, (f if n in _keep else set())) for (n, f) in _tables]

    def _patched_insert_act_table_loads():
        _bass_rust.insert_act_table_loads(nc, _tables)

    nc.insert_act_table_loads = _patched_insert_act_table_loads
    nc.compile()
    return nc


def prep_shared(inputs):
    """Fold LN affine params / scale constants into weights -> shared in_map entries."""
    bf = ml_dtypes.bfloat16
    f32 = np.float32
    g = {k: np.asarray(v, dtype=np.float32) for k, v in inputs.items() if k != "x"}

    # projection/MLP weights in fp8 e4m3, scaled 16x so the smallest weights
    # stay out of the subnormal floor; the 1/16 is folded into the PSUM
    # evacuations (tensor_scalar mult / SILU input scale).
    e4 = ml_dtypes.float8_e4m3
    wqT = (g["ln1_w"][:, None] * g["qw"].T * 0.125 * 16.0).astype(e4)
    wkT = (g["ln1_w"][:, None] * g["kw"].T * 16.0).astype(e4)
    wvT = (g["ln1_w"][:, None] * g["vw"].T * 16.0).astype(e4)
    woT = np.ascontiguousarray(g["ow"].T).astype(bf)
    fc1T = (g["ln2_w"][:, None] * g["fc1_w"].T * 16.0).astype(e4)
    fc2T = (g["fc2_w"].T / 1.702 * 16.0).astype(e4)

    qb = ((g["ln1_b"] @ g["qw"].T + g["qb"]) * 0.125).astype(f32)
    kb = (g["ln1_b"] @ g["kw"].T + g["kb"]).astype(f32)
    vb = (g["ln1_b"] @ g["vw"].T + g["vb"]).astype(f32)
    ob = g["ob"].astype(f32)
    fc1b = ((g["ln2_b"] @ g["fc1_w"].T + g["fc1_b"]) * 1.702).astype(f32)
    fc2b = g["fc2_b"].astype(f32)

    # additive causal mask in scoresT[k, q] orientation (k > q masked),
    # tiled 6x along q for the per-bank [77, 462] accumulation matmul.
    m1 = np.where(np.arange(S)[:, None] > np.arange(S)[None, :], MASK_NEG, 0.0)
    mask6 = np.tile(m1.astype(np.float32), (1, H // 2)).astype(bf)

    return dict(wqT=wqT, wkT=wkT, wvT=wvT, woT=woT, fc1T=fc1T, fc2T=fc2T,
                qb=qb, kb=kb, vb=vb, ob=ob, fc1b=fc1b, fc2b=fc2b, mask6=mask6)


def prep_host_inputs(inputs):
    shared = prep_shared(inputs)
    x = np.asarray(inputs["x"], dtype=np.float32)
    in_maps = []
    for c in range(N_CORES):
        xc = np.ascontiguousarray(
            x[c * BPC:(c + 1) * BPC].reshape(T_CORE, D).astype(np.float32))
        in_maps.append(dict(shared, x=xc))
    return in_maps


_CACHED_NC = None


def _get_nc():
    global _CACHED_NC
    if _CACHED_NC is None:
        _CACHED_NC = build_program()
    return _CACHED_NC


def run(inputs, trace=False):
    from concourse.bass_utils import run_bass_kernel_spmd
    nc = _get_nc()
    in_maps = prep_host_inputs(inputs)
    res = run_bass_kernel_spmd(nc, in_maps, list(range(N_CORES)), trace=trace)
    outs = [np.asarray(res.results[c]["out"], dtype=np.float32).reshape(BPC, S, D)
            for c in range(N_CORES)]
    full = np.concatenate(outs, axis=0)
    return full, res


def kernel(**inputs):
    full, _ = run(inputs, trace=False)
    return full



# revision 35
# speedup vs baseline: 1.0528x; 1.0037x over previous
"""CLIP encoder layer on 8 trn2 NeuronCores, pure data parallel over batch.

Layout strategy (per core, batch shard of 64 sequences = 4928 tokens):
  - x arrives token-major [T, 768] fp32.
  - LayerNorm runs token-major (tokens on partitions, bn_stats/bn_aggr),
    LN scale/bias folded into the downstream projection weights host-side.
    rstd computed as exp(-0.5*ln(var+eps)) so LN shares the scalar-engine
    natural_log_exp table set with attention's EXP (no sqrt-set thrash).
  - Normalized activations are PE-transposed (bf16) to feature-major
    [768, N] for the projections (weights stationary, activations moving).
  - Attention per sequence (S=77): scoresT[k,q] = kT.T @ qT per head,
    written directly in k-major orientation so no post-softmax transpose is
    needed; 6 even heads share one PSUM bank, 6 odd heads another (row-group
    packed pairs run concurrently in the PE).  The causal mask is ADDED via
    an identity-matmul accumulation (-1e5 above diagonal) before EXP, so the
    only cross-engine dependency between scores and ctx is a single EXP per
    bank.  Softmax denominators come out of a ones-matrix matmul
    (ones77.T @ pT -> every partition holds the per-query denominator);
    normalization is fused into the ctx PSUM->SBUF evacuation multiply.
  - ctx matmuls are column-packed pairs writing [128, 77] PSUM tiles that
    are already in the ctxT feature-major layout the O-projection wants.
  - O-projection runs with swapped operands (activations stationary) so its
    output comes out token-major for the residual add (x+ob precomputed on
    the otherwise-idle GpSimd engine) and the second LayerNorm.
  - FC1 feature-major (weights stationary).  FC2 also feature-major
    (44K vs 55K PE cycles), then PE-transposed back to token-major with the
    final residual add fused into the PSUM evacuation.
  - All matmuls in bf16 (fp32 PSUM accumulation); fp32 elsewhere.
    QuickGELU via ACT Silu: x*sigmoid(1.702x) = silu(1.702x)/1.702 with the
    1/1.702 folded into fc2 weights and the 1.702 into the ACT input scale.

Scheduling (the big lever on this kernel -- the attention phase alone
leaves the PE idle/HAM-throttled):
  - The whole MLP (FC1+FC2) is software-pipelined ONE superblock behind
    attention and emitted at the lowest priority, so its dense fp8 matmul
    stream statically fills the attention-phase dependency stalls and the
    boundary transpose chains, keeping the PE's HAM clock-gate warm.
    (Its SILUs do interleave with attention's EXP/LN ops, costing ~11
    activation-table loads per SB on the scalar engine, but ACT has slack
    and the PE overlap is worth far more.)
  - The next superblock's x-loads + LN1 run mid-attention (vector/scalar
    work); only the PE transposes stay at the boundary, batched so
    transpose-mode switches stay clustered.
  - Transpose PSUM evacuations alternate scalar/vector (ACT Copy is in
    every table set); the softmax reciprocal's Ln/Exp run as single
    full-width [128,462] ops (ACT cost scales with free size only).
  - PSUM banks: tr+ctx share one 2-buf tag (their phases are temporally
    disjoint), qkv/v/o-proj share "big", scores "sc", FC1/FC2 "fc".
"""

import os
import numpy as np
import ml_dtypes

D = 768
H = 12
HD = 64
S = 77
FF = 3072
EPS = 1e-5
N_CORES = 8
B_FULL = 512
BPC = B_FULL // N_CORES          # 64 sequences per core
T_CORE = BPC * S                 # 4928 tokens per core
G_SEQ = 4                        # sequences per superblock
SB = G_SEQ * S                   # 308 tokens per superblock
MASK_NEG = -1.0e5                # additive causal mask (exp -> exact 0)


def build_program(T=T_CORE, G=G_SEQ):
    import concourse.bass as bass
    import concourse.bacc as bacc
    import concourse.mybir as mybir
    import concourse.tile as tile
    from concourse.masks import make_identity
    from contextlib import ExitStack

    f32 = mybir.dt.float32
    bf16 = mybir.dt.bfloat16
    AX = mybir.AxisListType
    OP = mybir.AluOpType
    AF = mybir.ActivationFunctionType

    SBLK = G * S
    NSB = T // SBLK
    assert NSB * SBLK == T
    NH2 = H // 2                  # 6 head pairs
    SW = NH2 * S                  # 462 score columns per bank
    # token chunks within a superblock
    chunks = []
    off = 0
    while off < SBLK:
        w = min(128, SBLK - off)
        chunks.append((off, w))
        off += w

    nc = bacc.Bacc("TRN2", target_bir_lowering=False)

    fp8 = mybir.dt.float8e4
    x_d = nc.declare_dram_parameter("x", [T, D], f32, isOutput=False)
    wq_d = nc.declare_dram_parameter("wqT", [D, D], fp8, isOutput=False)
    wk_d = nc.declare_dram_parameter("wkT", [D, D], fp8, isOutput=False)
    wv_d = nc.declare_dram_parameter("wvT", [D, D], fp8, isOutput=False)
    wo_d = nc.declare_dram_parameter("woT", [D, D], bf16, isOutput=False)
    wf1_d = nc.declare_dram_parameter("fc1T", [D, FF], fp8, isOutput=False)
    wf2_d = nc.declare_dram_parameter("fc2T", [FF, D], fp8, isOutput=False)
    qb_d = nc.declare_dram_parameter("qb", [D], f32, isOutput=False)
    kb_d = nc.declare_dram_parameter("kb", [D], f32, isOutput=False)
    vb_d = nc.declare_dram_parameter("vb", [D], f32, isOutput=False)
    ob_d = nc.declare_dram_parameter("ob", [D], f32, isOutput=False)
    f1b_d = nc.declare_dram_parameter("fc1b", [FF], f32, isOutput=False)
    f2b_d = nc.declare_dram_parameter("fc2b", [D], f32, isOutput=False)
    mask_d = nc.declare_dram_parameter("mask6", [S, SW], bf16, isOutput=False)
    out_d = nc.declare_dram_parameter("out", [T, D], f32, isOutput=True)

    with tile.TileContext(nc) as tc, ExitStack() as ctx:
        singles = ctx.enter_context(tc.tile_pool(name="singles", bufs=1))
        xpool = ctx.enter_context(tc.tile_pool(name="xpool", bufs=5))
        # x2 residual tiles must survive one extra superblock (stage F is
        # software-pipelined one SB behind attention): 3 chunks x 2 SBs.
        x2pool = ctx.enter_context(tc.tile_pool(name="x2pool", bufs=6))
        actpool = ctx.enter_context(tc.tile_pool(name="actpool", bufs=1))
        outpool = ctx.enter_context(tc.tile_pool(name="outpool", bufs=1))
        attnpool = ctx.enter_context(tc.tile_pool(name="attnpool", bufs=2))
        statpool = ctx.enter_context(tc.tile_pool(name="statpool", bufs=2))
        pspool = ctx.enter_context(tc.tile_pool(name="pspool", bufs=2, space="PSUM"))

        # ---- constants / weights ----
        wq_sb = singles.tile([128, D // 128, D], fp8)
        wk_sb = singles.tile([128, D // 128, D], fp8)
        wv_sb = singles.tile([128, D // 128, D], fp8)
        wo_sb = singles.tile([128, D // 128, D], bf16)
        wf1_sb = singles.tile([128, D // 128, FF], fp8)
        wf2_sb = singles.tile([128, FF // 128, D], fp8)
        for sb_t, dr in ((wq_sb, wq_d), (wk_sb, wk_d), (wv_sb, wv_d),
                         (wo_sb, wo_d), (wf1_sb, wf1_d), (wf2_sb, wf2_d)):
            nc.sync.dma_start(out=sb_t, in_=dr[:].rearrange("(c p) o -> p c o", p=128))

        qb_sb = singles.tile([128, D // 128], f32)
        kb_sb = singles.tile([128, D // 128], f32)
        f1b_sb = singles.tile([128, FF // 128], f32)
        f2b_sb = singles.tile([128, D // 128], f32)
        for sb_t, dr in ((qb_sb, qb_d), (kb_sb, kb_d), (f1b_sb, f1b_d),
                         (f2b_sb, f2b_d)):
            nc.sync.dma_start(out=sb_t, in_=dr[:].rearrange("(c p) -> p c", p=128))

        # free-axis biases broadcast to all 128 partitions
        ob_bc = singles.tile([128, D], f32)
        vb_bc = singles.tile([128, D], f32)
        f2b_bc = singles.tile([128, D], f32)
        for sb_t, dr in ((ob_bc, ob_d), (vb_bc, vb_d), (f2b_bc, f2b_d)):
            src = bass.AP(tensor=dr[:].tensor, offset=dr[:].offset,
                          ap=[[0, 128]] + list(dr[:].ap))
            nc.sync.dma_start(out=sb_t, in_=src)

        mask6_sb = singles.tile([S, SW], bf16)
        nc.sync.dma_start(out=mask6_sb, in_=mask_d[:])

        ident = singles.tile([128, 128], bf16)
        make_identity(nc, ident)

        ones77 = singles.tile([S, 128], bf16)
        nc.vector.memset(ones77, 1.0)

        eps_sb = singles.tile([128, 1], f32)
        nc.vector.memset(eps_sb, EPS)

        NCH = D // 128    # 6
        NFF = FF // 128   # 24

        def ln_normalize(src_tile, w, tag, bufs=2):
            """token-major [w, 768] fp32 -> normalized bf16 htok tile."""
            stats = statpool.tile([128, 3, 6], f32, tag=f"stats{tag}", name=f"stats{tag}")
            mv = statpool.tile([128, 3], f32, tag=f"mv{tag}", name=f"mv{tag}")
            xg = src_tile[:w].rearrange("p (s f) -> p s f", f=256)
            for i in range(3):
                nc.vector.bn_stats(out=stats[:w, i, :], in_=xg[:, i, :])
            nc.vector.bn_aggr(out=mv[:w, 0:2], in_=stats[:w])
            mean = mv[:w, 0:1]
            var = mv[:w, 1:2]
            lnv = mv[:w, 2:3]
            # rstd = exp(-0.5*ln(var+eps)) - stays in the natural_log_exp
            # activation-table set shared with attention's EXP.
            nc.scalar.activation(out=lnv, in_=var, func=AF.Ln,
                                 bias=eps_sb[:w], scale=1.0)
            nc.scalar.activation(out=var, in_=lnv, func=AF.Exp,
                                 bias=0.0, scale=-0.5)
            rstd = var
            htok = statpool.tile([128, D], bf16, tag=f"htok{tag}", name=f"htok{tag}",
                                 bufs=bufs)
            nc.vector.tensor_scalar(out=htok[:w], in0=src_tile[:w],
                                    scalar1=mean, scalar2=rstd,
                                    op0=OP.subtract, op1=OP.mult)
            return htok

        def seq_pieces(coff, w):
            """split token range [coff, coff+w) into per-sequence pieces of
            (src_col_in_chunk, length, dst_col) with 80-padded dst stride."""
            out_ = []
            t = coff
            while t < coff + w:
                s_ = t // S
                e = min((s_ + 1) * S, coff + w)
                out_.append((t - coff, e - t, s_ * 80 + (t - s_ * S)))
                t = e
            return out_

        def ln_transpose(htok, coff, w, dst, tag, padded=False):
            pieces = seq_pieces(coff, w) if padded else [(0, w, coff)]
            for c in range(NCH):
                ps = pspool.tile([128, 128], bf16, tag="tr", name=f"trp{tag}",
                                 bufs=2)
                nc.tensor.transpose(ps[:, :w], htok[:w, c * 128:(c + 1) * 128],
                                    ident[:w, :w])
                # evacuations alternate scalar/vector so neither in-order
                # engine queue serializes the transpose chain (ACT Copy is in
                # every activation table set -> no table loads).
                for (po_, ln_, dc_) in pieces:
                    if c % 2 == 0:
                        nc.scalar.activation(out=dst(c, dc_, ln_),
                                             in_=ps[:, po_:po_ + ln_],
                                             func=AF.Copy)
                    else:
                        nc.vector.tensor_copy(out=dst(c, dc_, ln_),
                                              in_=ps[:, po_:po_ + ln_])

        def stage_A_ln(isb):
            """load x + LN1 (vector/scalar work, emitted mid-attention of the
            previous SB so the normalized htok tiles are ready before the
            boundary); then x_tok += ob in place (idle GpSimd engine)."""
            t0 = isb * SBLK
            x_tiles = []
            htoks = []
            for (coff, w) in chunks:
                x_tok = xpool.tile([128, D], f32, tag="xtok", name="xtok")
                nc.sync.dma_start(out=x_tok[:w], in_=x_d[t0 + coff: t0 + coff + w, :])
                x_tiles.append(x_tok)
                htoks.append(ln_normalize(x_tok, w, "A", bufs=3))
                # after LN consumed raw x: fold the o-proj bias into the
                # residual in place on the idle GpSimd engine.
                nc.gpsimd.tensor_tensor(out=x_tok[:w], in0=x_tok[:w],
                                        in1=ob_bc[:w], op=OP.add)
            return htoks, x_tiles

        def stage_A_tr(htoks):
            """PE-transpose LN1 output to the feature-major fp8 layout (kept
            at the SB boundary so transpose-mode switches stay clustered)."""
            hT8 = actpool.tile([128, NCH, 320], fp8, tag="hT8", name="hT8", bufs=2)
            for ci, (coff, w) in enumerate(chunks):
                ln_transpose(htoks[ci], coff, w,
                             lambda c, o, ww: hT8[:, c, o:o + ww], "A",
                             padded=True)
            return hT8

        def stage_D_chunk(ci, ctxT, x_tiles, x2_tiles, x2f_tiles):
            coff, w = chunks[ci]
            x2 = x2pool.tile([128, D], f32, tag="x2tok", name="x2tok")
            for half in range(2):
                ps = pspool.tile([128, 384], f32, tag="big", name="pso")
                for d in range(NCH):
                    nc.tensor.matmul(ps[:w], lhsT=ctxT[d][:, coff:coff + w],
                                     rhs=wo_sb[:, d, half * 384:(half + 1) * 384],
                                     start=(d == 0), stop=(d == NCH - 1))
                sl = slice(half * 384, (half + 1) * 384)
                nc.vector.tensor_tensor(out=x2[:w, sl], in0=ps[:w],
                                        in1=x_tiles[ci][:w, sl], op=OP.add)
            x2_tiles.append(x2)

        DR = mybir.MatmulPerfMode.DoubleRow

        def stage_FC1(h2T8):
            """FC1 + QuickGELU, emitted as a dense block at the superblock
            boundary so its SILUs stay contiguous on the scalar engine (one
            silu-table load per superblock, no exp<->silu thrash)."""
            ff1_8 = actpool.tile([128, NFF, 320], fp8, tag="ff18", name="ff18",
                                 bufs=2)
            for f in range(NFF):
                ps = pspool.tile([128, SBLK], f32, tag="fc", name="psff")
                for dp in range(NCH // 2):
                    nc.tensor.matmul(ps,
                                     lhsT=wf1_sb[:, 2 * dp:2 * dp + 2,
                                                 f * 128:(f + 1) * 128],
                                     rhs=h2T8[:, 2 * dp:2 * dp + 2, :SBLK],
                                     perf_mode=DR,
                                     start=(dp == 0), stop=(dp == NCH // 2 - 1))
                # f1 = silu(1.702*(ps/16) + 1.702*b) = 1.702*quickgelu(ps+b);
                # the 1/1.702 is folded into fc2T host-side.
                nc.scalar.activation(out=ff1_8[:, f, :SBLK], in_=ps, func=AF.Silu,
                                     bias=f1b_sb[:, f:f + 1], scale=1.702 / 16)
            return ff1_8

        def stage_FC2(t0, ff1_8, x2_tiles):
            """FC2 + final residual + store, software-pipelined one SB behind.

            Pure PE/vector work (no scalar-engine activations), so the
            scheduler can interleave its dense fp8 matmuls into the next
            superblock's attention-region PE stalls -- filling dependency
            gaps and keeping the PE HAM clock-gate warm -- without
            perturbing the exp/silu activation-table sequence.
            """
            o_toks = [outpool.tile([128, D], f32, tag=f"otok{ci}", name=f"otok{ci}")
                      for ci in range(len(chunks))]
            for c in range(NCH):
                ps = pspool.tile([128, SBLK], f32, tag="fc", name="psf2")
                for fp in range(NFF // 2):
                    nc.tensor.matmul(ps,
                                     lhsT=wf2_sb[:, 2 * fp:2 * fp + 2,
                                                 c * 128:(c + 1) * 128],
                                     rhs=ff1_8[:, 2 * fp:2 * fp + 2, :SBLK],
                                     perf_mode=DR,
                                     start=(fp == 0), stop=(fp == NFF // 2 - 1))
                x3c = statpool.tile([128, SBLK], bf16, tag="x3", name="x3")
                nc.vector.tensor_scalar(out=x3c, in0=ps,
                                        scalar1=1.0 / 16, scalar2=f2b_sb[:, c:c + 1],
                                        op0=OP.mult, op1=OP.add)
                for ci, (coff, w) in enumerate(chunks):
                    tr = pspool.tile([128, 128], bf16, tag="fc", name="trf")
                    nc.tensor.transpose(tr[:w, :], x3c[:, coff:coff + w], ident)
                    nc.vector.tensor_tensor(
                        out=o_toks[ci][:w, c * 128:(c + 1) * 128],
                        in0=tr[:w, :],
                        in1=x2_tiles[ci][:w, c * 128:(c + 1) * 128], op=OP.add)
            for ci, (coff, w) in enumerate(chunks):
                nc.sync.dma_start(out=out_d[t0 + coff: t0 + coff + w, :],
                                  in_=o_toks[ci][:w])

        a0 = stage_A_ln(0)
        cur = (stage_A_tr(a0[0]), a0[1])
        next_ln = None
        fc_pending = None
        for isb in range(NSB):
            t0 = isb * SBLK
            hT8, x_tiles = cur

            # ---- stage B: q/k projections (fp8 DoubleRow, weights 16x).
            # qT/kT inherit hT8's 80-padded per-sequence column layout. ----
            qT = [actpool.tile([128, 320], bf16, tag=f"qT{c}", name=f"qT{c}")
                  for c in range(NCH)]
            kT = [actpool.tile([128, 320], bf16, tag=f"kT{c}", name=f"kT{c}")
                  for c in range(NCH)]
            for dst, w_sb, b_sb in ((qT, wq_sb, qb_sb), (kT, wk_sb, kb_sb)):
                for c in range(NCH):
                    ps = pspool.tile([128, 320], f32, tag="big", name="psqkv")
                    for dp in range(NCH // 2):
                        nc.tensor.matmul(ps,
                                         lhsT=w_sb[:, 2 * dp:2 * dp + 2,
                                                   c * 128:(c + 1) * 128],
                                         rhs=hT8[:, 2 * dp:2 * dp + 2, :],
                                         perf_mode=DR,
                                         start=(dp == 0), stop=(dp == NCH // 2 - 1))
                    nc.vector.tensor_scalar(out=dst[c], in0=ps,
                                            scalar1=1.0 / 16,
                                            scalar2=b_sb[:, c:c + 1],
                                            op0=OP.mult, op1=OP.add)

            # ---- stage C: attention per sequence ----
            ctxT = [actpool.tile([128, SBLK], bf16, tag=f"ctxT{c}", name=f"ctxT{c}",
                                 bufs=2)
                    for c in range(NCH)]
            # fp8 feature-major LN2 activations for the DoubleRow FC1
            # (padded to 320 so the k-pair free step is 16B-aligned);
            # double-buffered: stage F consumes it one superblock later.
            h2T8 = actpool.tile([128, NCH, 320], fp8, tag="h2T8", name="h2T8",
                                bufs=2)
            x2_tiles = []
            x2f_tiles = []
            h2toks = []
            next_chunk = 0
            for s in range(G):
                so = s * S        # token-contiguous column base (ctxT)
                sp = s * 80       # 80-padded column base (hT8/qT/kT)
                # v for this sequence, token-major directly (swapped operands,
                # fp8 DoubleRow; the padded hT8 base keeps offsets 16B-aligned)
                vtok = attnpool.tile([S, H, HD], bf16, tag="vtok", name="vtok")
                for half in range(2):
                    psv = pspool.tile([S, 384], f32, tag="big", name="psvtok")
                    for dp in range(NCH // 2):
                        nc.tensor.matmul(psv,
                                         lhsT=hT8[:, 2 * dp:2 * dp + 2,
                                                  sp:sp + S],
                                         rhs=wv_sb[:, 2 * dp:2 * dp + 2,
                                                   half * 384:(half + 1) * 384],
                                         perf_mode=DR,
                                         start=(dp == 0), stop=(dp == NCH // 2 - 1))
                    nc.vector.scalar_tensor_tensor(
                        out=vtok[:, half * 6:(half + 1) * 6, :],
                        in0=psv, scalar=1.0 / 16,
                        in1=vb_bc[:S, half * 384:(half + 1) * 384],
                        op0=OP.mult, op1=OP.add)
                # scoresT[k, q] per head; even heads -> bank A, odd -> bank B.
                # Row-group packing: even heads live on partitions 0:64 of
                # their qT/kT chunk, odd heads on 64:128 -> pairs overlap.
                psc = [pspool.tile([S, SW], f32, tag="sc", name=f"psc{a}")
                       for a in range(2)]
                for j in range(NH2):
                    for a in range(2):
                        h = 2 * j + a
                        c, po = h // 2, 64 * (h % 2)
                        nc.tensor.matmul(psc[a][:, j * S:(j + 1) * S],
                                         lhsT=kT[c][po:po + 64, sp:sp + S],
                                         rhs=qT[c][po:po + 64, sp:sp + S],
                                         start=(j == 0), stop=False,
                                         skip_group_check=True)
                # additive causal mask via identity-matmul accumulation
                pT = attnpool.tile([S, 2, SW], bf16, tag="pT", name="pT")
                for a in range(2):
                    nc.tensor.matmul(psc[a], lhsT=ident[:S, :S], rhs=mask6_sb,
                                     start=False, stop=True, skip_group_check=True)
                    nc.scalar.activation(out=pT[:, a, :], in_=psc[a], func=AF.Exp)
                # denominators broadcast across partitions: ones.T @ pT, the
                # two banks column-packed into one PSUM tile (concurrent).
                dben = pspool.tile([128, SW], f32, tag="sc", name="dben")
                rp = attnpool.tile([128, SW], bf16, tag="rp", name="rp")
                lnd = attnpool.tile([128, SW], f32, tag="lnd", name="lnd")
                for a in range(2):
                    nc.tensor.matmul(dben[64 * a:64 * a + 64, :],
                                     lhsT=ones77[:, 64 * a:64 * a + 64],
                                     rhs=pT[:, a, :],
                                     start=True, stop=True,
                                     skip_group_check=True)
                # reciprocal as exp(-ln(x)) on the scalar engine: same table
                # set as the attention EXP, and off the busy vector engine
                # (nc.vector.reciprocal is ~6.5ns/elem - 3us per call here).
                # One full-width op per step: ACT cost scales with free dim
                # only, so [128,462] costs the same as [64,462].
                nc.scalar.activation(out=lnd, in_=dben, func=AF.Ln)
                nc.scalar.activation(out=rp, in_=lnd, func=AF.Exp, scale=-1.0)
                # ctx per head pair, column-packed into [128, 77] PSUM already
                # in ctxT layout; normalization fused into the evacuation.
                for j in range(NH2):
                    ctxp = pspool.tile([128, S], f32, tag="tr", name="ctxp",
                                       bufs=2)
                    for a in range(2):
                        h = 2 * j + a
                        nc.tensor.matmul(ctxp[64 * a:64 * a + 64, :],
                                         lhsT=vtok[:, h, :],
                                         rhs=pT[:, a, j * S:(j + 1) * S],
                                         start=True, stop=True,
                                         skip_group_check=True)
                    nc.vector.tensor_tensor(out=ctxT[j][:, so:so + S], in0=ctxp,
                                            in1=rp[:, j * S:(j + 1) * S],
                                            op=OP.mult)
                # prefetch next SB's x-loads + LN1 mid-attention (vector/
                # scalar work only; the PE transposes stay at the boundary)
                if s == 1 and isb + 1 < NSB:
                    next_ln = stage_A_ln(isb + 1)

                # emit O-proj + residual + LN2 for chunks fully covered
                done_tokens = (s + 1) * S
                while (next_chunk < len(chunks)
                       and chunks[next_chunk][0] + chunks[next_chunk][1]
                       <= done_tokens):
                    ci = next_chunk
                    stage_D_chunk(ci, ctxT, x_tiles, x2_tiles, x2f_tiles)
                    coff, w = chunks[ci]
                    h2toks.append(ln_normalize(x2_tiles[ci], w, "E", bufs=3))
                    next_chunk += 1

            # ---- next superblock's LN1 transposes (LN itself already ran
            # mid-attention); clustered here so the PE's transpose-mode
            # switches stay batched ----
            if isb + 1 < NSB:
                cur = (stage_A_tr(next_ln[0]), next_ln[1])

            # E transposes (emitted after C so the in-order PE isn't blocked
            # mid-attention waiting on the LN chains; batched here so the
            # PE's transpose-mode switches stay clustered)
            for ci, (coff, w) in enumerate(chunks):
                ln_transpose(h2toks[ci], coff, w,
                             lambda c, o, ww: h2T8[:, c, o:o + ww], "E")

            # ---- the WHOLE MLP of the previous superblock (FC1 then FC2)
            # emitted here at low priority: FC1(i-1) is ready from the start
            # of this SB, so the filler stream covers this SB's attention
            # stalls, and FC2(i-1) remnants are still in flight to cover
            # the boundary transpose chains.  (FC1's silu<->exp table loads
            # cost ~1.3us each on the scalar engine, but ACT has slack and
            # the PE overlap is worth more.) ----
            if fc_pending is not None:
                pt0, ph2T8, px2 = fc_pending
                ff1_8 = stage_FC1(ph2T8)
                stage_FC2(pt0, ff1_8, px2)
            fc_pending = (t0, h2T8, x2_tiles)
        pt0, ph2T8, px2 = fc_pending
        ff1_8 = stage_FC1(ph2T8)
        stage_FC2(pt0, ff1_8, px2)

    # Restrict the activation-table-set chooser to the two sets that cover
    # everything this kernel uses (ln+exp share one set; silu the other).
    # Entries keep their original indices (act_func_set_id is positional);
    # unwanted sets are just emptied so the chooser can never pick them.
    from concourse.hw_specs import get_activation_tables
    import bass_rust as _bass_rust
    _tables = list(get_activation_tables(nc.m.arch).items())
    _keep = {"natural_log_exp_and_others", "silu_and_others"}
    _tables = [(n, (f if n in _keep else set())) for (n, f) in _tables]

    def _patched_insert_act_table_loads():
        _bass_rust.insert_act_table_loads(nc, _tables)

    nc.insert_act_table_loads = _patched_insert_act_table_loads
    nc.compile()
    return nc


def prep_shared(inputs):
    """Fold LN affine params / scale constants into weights -> shared in_map entries."""
    bf = ml_dtypes.bfloat16
    f32 = np.float32
    g = {k: np.asarray(v, dtype=np.float32) for k, v in inputs.items() if k != "x"}

    # projection/MLP weights in fp8 e4m3, scaled 16x so the smallest weights
    # stay out of the subnormal floor; the 1/16 is folded into the PSUM
    # evacuations (tensor_scalar mult / SILU input scale).
    e4 = ml_dtypes.float8_e4m3
    wqT = (g["ln1_w"][:, None] * g["qw"].T * 0.125 * 16.0).astype(e4)
    wkT = (g["ln1_w"][:, None] * g["kw"].T * 16.0).astype(e4)
    wvT = (g["ln1_w"][:, None] * g["vw"].T * 16.0).astype(e4)
    woT = np.ascontiguousarray(g["ow"].T).astype(bf)
    fc1T = (g["ln2_w"][:, None] * g["fc1_w"].T * 16.0).astype(e4)
    fc2T = (g["fc2_w"].T / 1.702 * 16.0).astype(e4)

    qb = ((g["ln1_b"] @ g["qw"].T + g["qb"]) * 0.125).astype(f32)
    kb = (g["ln1_b"] @ g["kw"].T + g["kb"]).astype(f32)
    vb = (g["ln1_b"] @ g["vw"].T + g["vb"]).astype(f32)
    ob = g["ob"].astype(f32)
    fc1b = ((g["ln2_b"] @ g["fc1_w"].T + g["fc1_b"]) * 1.702).astype(f32)
    fc2b = g["fc2_b"].astype(f32)

    # additive causal mask in scoresT[k, q] orientation (k > q masked),
    # tiled 6x along q for the per-bank [77, 462] accumulation matmul.
    m1 = np.where(np.arange(S)[:, None] > np.arange(S)[None, :], MASK_NEG, 0.0)
    mask6 = np.tile(m1.astype(np.float32), (1, H // 2)).astype(bf)

    return dict(wqT=wqT, wkT=wkT, wvT=wvT, woT=woT, fc1T=fc1T, fc2T=fc2T,
                qb=qb, kb=kb, vb=vb, ob=ob, fc1b=fc1b, fc2b=fc2b, mask6=mask6)


def prep_host_inputs(inputs):
    shared = prep_shared(inputs)
    x = np.asarray(inputs["x"], dtype=np.float32)
    in_maps = []
    for c in range(N_CORES):
        xc = np.ascontiguousarray(
            x[c * BPC:(c + 1) * BPC].reshape(T_CORE, D).astype(np.float32))
        in_maps.append(dict(shared, x=xc))
    return in_maps


_CACHED_NC = None


def _get_nc():
    global _CACHED_NC
    if _CACHED_NC is None:
        _CACHED_NC = build_program()
    return _CACHED_NC


def run(inputs, trace=False):
    from concourse.bass_utils import run_bass_kernel_spmd
    nc = _get_nc()
    in_maps = prep_host_inputs(inputs)
    res = run_bass_kernel_spmd(nc, in_maps, list(range(N_CORES)), trace=trace)
    outs = [np.asarray(res.results[c]["out"], dtype=np.float32).reshape(BPC, S, D)
            for c in range(N_CORES)]
    full = np.concatenate(outs, axis=0)
    return full, res


def kernel(**inputs):
    full, _ = run(inputs, trace=False)
    return full



# revision 36
# speedup vs baseline: 1.0580x; 1.0049x over previous
"""CLIP encoder layer on 8 trn2 NeuronCores, pure data parallel over batch.

Layout strategy (per core, batch shard of 64 sequences = 4928 tokens):
  - x arrives token-major [T, 768] fp32.
  - LayerNorm runs token-major (tokens on partitions, bn_stats/bn_aggr),
    LN scale/bias folded into the downstream projection weights host-side.
    rstd computed as exp(-0.5*ln(var+eps)) so LN shares the scalar-engine
    natural_log_exp table set with attention's EXP (no sqrt-set thrash).
  - Normalized activations are PE-transposed (bf16) to feature-major
    [768, N] for the projections (weights stationary, activations moving).
  - Attention per sequence (S=77): scoresT[k,q] = kT.T @ qT per head,
    written directly in k-major orientation so no post-softmax transpose is
    needed; 6 even heads share one PSUM bank, 6 odd heads another (row-group
    packed pairs run concurrently in the PE).  The causal mask is ADDED via
    an identity-matmul accumulation (-1e5 above diagonal) before EXP, so the
    only cross-engine dependency between scores and ctx is a single EXP per
    bank.  Softmax denominators come out of a ones-matrix matmul
    (ones77.T @ pT -> every partition holds the per-query denominator);
    normalization is fused into the ctx PSUM->SBUF evacuation multiply.
  - ctx matmuls are column-packed pairs writing [128, 77] PSUM tiles that
    are already in the ctxT feature-major layout the O-projection wants.
  - O-projection runs with swapped operands (activations stationary) so its
    output comes out token-major for the residual add (x+ob precomputed on
    the otherwise-idle GpSimd engine) and the second LayerNorm.
  - FC1 feature-major (weights stationary).  FC2 also feature-major
    (44K vs 55K PE cycles), then PE-transposed back to token-major with the
    final residual add fused into the PSUM evacuation.
  - All matmuls in bf16 (fp32 PSUM accumulation); fp32 elsewhere.
    QuickGELU via ACT Silu: x*sigmoid(1.702x) = silu(1.702x)/1.702 with the
    1/1.702 folded into fc2 weights and the 1.702 into the ACT input scale.

Scheduling (the big lever on this kernel -- the attention phase alone
leaves the PE idle/HAM-throttled):
  - The whole MLP (FC1+FC2) is software-pipelined ONE superblock behind
    attention and emitted at the lowest priority, so its dense fp8 matmul
    stream statically fills the attention-phase dependency stalls and the
    boundary transpose chains, keeping the PE's HAM clock-gate warm.
    (Its SILUs do interleave with attention's EXP/LN ops, costing ~11
    activation-table loads per SB on the scalar engine, but ACT has slack
    and the PE overlap is worth far more.)
  - The next superblock's x-loads + LN1 run mid-attention (vector/scalar
    work); only the PE transposes stay at the boundary, batched so
    transpose-mode switches stay clustered.
  - Transpose PSUM evacuations alternate scalar/vector (ACT Copy is in
    every table set); the softmax reciprocal's Ln/Exp run as single
    full-width [128,462] ops (ACT cost scales with free size only).
  - PSUM banks: tr+ctx share one 2-buf tag (their phases are temporally
    disjoint), qkv/v/o-proj share "big", scores "sc", FC1/FC2 "fc".
"""

import os
import numpy as np
import ml_dtypes

D = 768
H = 12
HD = 64
S = 77
FF = 3072
EPS = 1e-5
N_CORES = 8
B_FULL = 512
BPC = B_FULL // N_CORES          # 64 sequences per core
T_CORE = BPC * S                 # 4928 tokens per core
G_SEQ = 4                        # sequences per superblock
SB = G_SEQ * S                   # 308 tokens per superblock
MASK_NEG = -1.0e5                # additive causal mask (exp -> exact 0)


def build_program(T=T_CORE, G=G_SEQ):
    import concourse.bass as bass
    import concourse.bacc as bacc
    import concourse.mybir as mybir
    import concourse.tile as tile
    from concourse.masks import make_identity
    from contextlib import ExitStack

    f32 = mybir.dt.float32
    bf16 = mybir.dt.bfloat16
    AX = mybir.AxisListType
    OP = mybir.AluOpType
    AF = mybir.ActivationFunctionType

    SBLK = G * S
    NSB = T // SBLK
    assert NSB * SBLK == T
    NH2 = H // 2                  # 6 head pairs
    SW = NH2 * S                  # 462 score columns per bank
    # token chunks within a superblock
    chunks = []
    off = 0
    while off < SBLK:
        w = min(128, SBLK - off)
        chunks.append((off, w))
        off += w

    nc = bacc.Bacc("TRN2", target_bir_lowering=False)

    fp8 = mybir.dt.float8e4
    x_d = nc.declare_dram_parameter("x", [T, D], f32, isOutput=False)
    wq_d = nc.declare_dram_parameter("wqT", [D, D], fp8, isOutput=False)
    wk_d = nc.declare_dram_parameter("wkT", [D, D], fp8, isOutput=False)
    wv_d = nc.declare_dram_parameter("wvT", [D, D], fp8, isOutput=False)
    wo_d = nc.declare_dram_parameter("woT", [D, D], bf16, isOutput=False)
    wf1_d = nc.declare_dram_parameter("fc1T", [D, FF], fp8, isOutput=False)
    wf2_d = nc.declare_dram_parameter("fc2T", [FF, D], fp8, isOutput=False)
    qb_d = nc.declare_dram_parameter("qb", [D], f32, isOutput=False)
    kb_d = nc.declare_dram_parameter("kb", [D], f32, isOutput=False)
    vb_d = nc.declare_dram_parameter("vb", [D], f32, isOutput=False)
    ob_d = nc.declare_dram_parameter("ob", [D], f32, isOutput=False)
    f1b_d = nc.declare_dram_parameter("fc1b", [FF], f32, isOutput=False)
    f2b_d = nc.declare_dram_parameter("fc2b", [D], f32, isOutput=False)
    mask_d = nc.declare_dram_parameter("mask6", [S, SW], bf16, isOutput=False)
    out_d = nc.declare_dram_parameter("out", [T, D], f32, isOutput=True)

    with tile.TileContext(nc) as tc, ExitStack() as ctx:
        singles = ctx.enter_context(tc.tile_pool(name="singles", bufs=1))
        xpool = ctx.enter_context(tc.tile_pool(name="xpool", bufs=5))
        # x2 residual tiles must survive one extra superblock (stage F is
        # software-pipelined one SB behind attention): 3 chunks x 2 SBs.
        x2pool = ctx.enter_context(tc.tile_pool(name="x2pool", bufs=6))
        actpool = ctx.enter_context(tc.tile_pool(name="actpool", bufs=1))
        outpool = ctx.enter_context(tc.tile_pool(name="outpool", bufs=1))
        attnpool = ctx.enter_context(tc.tile_pool(name="attnpool", bufs=2))
        statpool = ctx.enter_context(tc.tile_pool(name="statpool", bufs=2))
        pspool = ctx.enter_context(tc.tile_pool(name="pspool", bufs=2, space="PSUM"))

        # ---- constants / weights ----
        wq_sb = singles.tile([128, D // 128, D], fp8)
        wk_sb = singles.tile([128, D // 128, D], fp8)
        wv_sb = singles.tile([128, D // 128, D], fp8)
        wo_sb = singles.tile([128, D // 128, D], bf16)
        wf1_sb = singles.tile([128, D // 128, FF], fp8)
        wf2_sb = singles.tile([128, FF // 128, D], fp8)
        for sb_t, dr in ((wq_sb, wq_d), (wk_sb, wk_d), (wv_sb, wv_d),
                         (wo_sb, wo_d), (wf1_sb, wf1_d), (wf2_sb, wf2_d)):
            nc.sync.dma_start(out=sb_t, in_=dr[:].rearrange("(c p) o -> p c o", p=128))

        qb_sb = singles.tile([128, D // 128], f32)
        kb_sb = singles.tile([128, D // 128], f32)
        f1b_sb = singles.tile([128, FF // 128], f32)
        f2b_sb = singles.tile([128, D // 128], f32)
        for sb_t, dr in ((qb_sb, qb_d), (kb_sb, kb_d), (f1b_sb, f1b_d),
                         (f2b_sb, f2b_d)):
            nc.sync.dma_start(out=sb_t, in_=dr[:].rearrange("(c p) -> p c", p=128))

        # free-axis biases broadcast to all 128 partitions
        ob_bc = singles.tile([128, D], f32)
        vb_bc = singles.tile([128, D], f32)
        f2b_bc = singles.tile([128, D], f32)
        for sb_t, dr in ((ob_bc, ob_d), (vb_bc, vb_d), (f2b_bc, f2b_d)):
            src = bass.AP(tensor=dr[:].tensor, offset=dr[:].offset,
                          ap=[[0, 128]] + list(dr[:].ap))
            nc.sync.dma_start(out=sb_t, in_=src)

        mask6_sb = singles.tile([S, SW], bf16)
        nc.sync.dma_start(out=mask6_sb, in_=mask_d[:])

        ident = singles.tile([128, 128], bf16)
        make_identity(nc, ident)

        ones77 = singles.tile([S, 128], bf16)
        nc.vector.memset(ones77, 1.0)

        eps_sb = singles.tile([128, 1], f32)
        nc.vector.memset(eps_sb, EPS)

        NCH = D // 128    # 6
        NFF = FF // 128   # 24

        def ln_normalize(src_tile, w, tag, bufs=2):
            """token-major [w, 768] fp32 -> normalized bf16 htok tile."""
            stats = statpool.tile([128, 3, 6], f32, tag=f"stats{tag}", name=f"stats{tag}")
            mv = statpool.tile([128, 3], f32, tag=f"mv{tag}", name=f"mv{tag}")
            xg = src_tile[:w].rearrange("p (s f) -> p s f", f=256)
            for i in range(3):
                nc.vector.bn_stats(out=stats[:w, i, :], in_=xg[:, i, :])
            nc.vector.bn_aggr(out=mv[:w, 0:2], in_=stats[:w])
            mean = mv[:w, 0:1]
            var = mv[:w, 1:2]
            lnv = mv[:w, 2:3]
            # rstd = exp(-0.5*ln(var+eps)) - stays in the natural_log_exp
            # activation-table set shared with attention's EXP.
            nc.scalar.activation(out=lnv, in_=var, func=AF.Ln,
                                 bias=eps_sb[:w], scale=1.0)
            nc.scalar.activation(out=var, in_=lnv, func=AF.Exp,
                                 bias=0.0, scale=-0.5)
            rstd = var
            htok = statpool.tile([128, D], bf16, tag=f"htok{tag}", name=f"htok{tag}",
                                 bufs=bufs)
            nc.vector.tensor_scalar(out=htok[:w], in0=src_tile[:w],
                                    scalar1=mean, scalar2=rstd,
                                    op0=OP.subtract, op1=OP.mult)
            return htok

        def seq_pieces(coff, w):
            """split token range [coff, coff+w) into per-sequence pieces of
            (src_col_in_chunk, length, dst_col) with 80-padded dst stride."""
            out_ = []
            t = coff
            while t < coff + w:
                s_ = t // S
                e = min((s_ + 1) * S, coff + w)
                out_.append((t - coff, e - t, s_ * 80 + (t - s_ * S)))
                t = e
            return out_

        def ln_transpose(htok, coff, w, dst, tag, padded=False):
            pieces = seq_pieces(coff, w) if padded else [(0, w, coff)]
            for c in range(NCH):
                ps = pspool.tile([128, 128], bf16, tag="tr", name=f"trp{tag}",
                                 bufs=2)
                nc.tensor.transpose(ps[:, :w], htok[:w, c * 128:(c + 1) * 128],
                                    ident[:w, :w])
                # evacuations alternate scalar/vector so neither in-order
                # engine queue serializes the transpose chain (ACT Copy is in
                # every activation table set -> no table loads).
                for (po_, ln_, dc_) in pieces:
                    if c % 2 == 0:
                        nc.scalar.activation(out=dst(c, dc_, ln_),
                                             in_=ps[:, po_:po_ + ln_],
                                             func=AF.Copy)
                    else:
                        nc.vector.tensor_copy(out=dst(c, dc_, ln_),
                                              in_=ps[:, po_:po_ + ln_])

        def stage_A_ln(isb):
            """load x + LN1 (vector/scalar work, emitted mid-attention of the
            previous SB so the normalized htok tiles are ready before the
            boundary); then x_tok += ob in place (idle GpSimd engine)."""
            t0 = isb * SBLK
            x_tiles = []
            htoks = []
            for (coff, w) in chunks:
                x_tok = xpool.tile([128, D], f32, tag="xtok", name="xtok")
                nc.sync.dma_start(out=x_tok[:w], in_=x_d[t0 + coff: t0 + coff + w, :])
                x_tiles.append(x_tok)
                htoks.append(ln_normalize(x_tok, w, "A", bufs=3))
                # after LN consumed raw x: fold the o-proj bias into the
                # residual in place on the idle GpSimd engine.
                nc.gpsimd.tensor_tensor(out=x_tok[:w], in0=x_tok[:w],
                                        in1=ob_bc[:w], op=OP.add)
            return htoks, x_tiles

        def stage_A_tr(htoks):
            """PE-transpose LN1 output to the feature-major fp8 layout (kept
            at the SB boundary so transpose-mode switches stay clustered)."""
            hT8 = actpool.tile([128, NCH, 320], fp8, tag="hT8", name="hT8", bufs=2)
            for ci, (coff, w) in enumerate(chunks):
                ln_transpose(htoks[ci], coff, w,
                             lambda c, o, ww: hT8[:, c, o:o + ww], "A",
                             padded=True)
            return hT8

        def stage_D_chunk(ci, ctxT, x_tiles, x2_tiles, x2f_tiles):
            coff, w = chunks[ci]
            x2 = x2pool.tile([128, D], f32, tag="x2tok", name="x2tok")
            for half in range(2):
                ps = pspool.tile([128, 384], f32, tag="big", name="pso")
                for d in range(NCH):
                    nc.tensor.matmul(ps[:w], lhsT=ctxT[d][:, coff:coff + w],
                                     rhs=wo_sb[:, d, half * 384:(half + 1) * 384],
                                     start=(d == 0), stop=(d == NCH - 1))
                sl = slice(half * 384, (half + 1) * 384)
                nc.vector.tensor_tensor(out=x2[:w, sl], in0=ps[:w],
                                        in1=x_tiles[ci][:w, sl], op=OP.add)
            x2_tiles.append(x2)

        DR = mybir.MatmulPerfMode.DoubleRow

        def stage_FC1(h2T8):
            """FC1 + QuickGELU, emitted as a dense block at the superblock
            boundary so its SILUs stay contiguous on the scalar engine (one
            silu-table load per superblock, no exp<->silu thrash)."""
            ff1_8 = actpool.tile([128, NFF, 320], fp8, tag="ff18", name="ff18",
                                 bufs=2)
            for f in range(NFF):
                ps = pspool.tile([128, SBLK], f32, tag="fc", name="psff")
                for dp in range(NCH // 2):
                    nc.tensor.matmul(ps,
                                     lhsT=wf1_sb[:, 2 * dp:2 * dp + 2,
                                                 f * 128:(f + 1) * 128],
                                     rhs=h2T8[:, 2 * dp:2 * dp + 2, :SBLK],
                                     perf_mode=DR,
                                     start=(dp == 0), stop=(dp == NCH // 2 - 1))
                # f1 = silu(1.702*(ps/16) + 1.702*b) = 1.702*quickgelu(ps+b);
                # the 1/1.702 is folded into fc2T host-side.
                nc.scalar.activation(out=ff1_8[:, f, :SBLK], in_=ps, func=AF.Silu,
                                     bias=f1b_sb[:, f:f + 1], scale=1.702 / 16)
            return ff1_8

        def stage_FC2(t0, ff1_8, x2_tiles):
            """FC2 + final residual + store, software-pipelined one SB behind.

            Pure PE/vector work (no scalar-engine activations), so the
            scheduler can interleave its dense fp8 matmuls into the next
            superblock's attention-region PE stalls -- filling dependency
            gaps and keeping the PE HAM clock-gate warm -- without
            perturbing the exp/silu activation-table sequence.
            """
            o_toks = [outpool.tile([128, D], f32, tag=f"otok{ci}", name=f"otok{ci}")
                      for ci in range(len(chunks))]
            for c in range(NCH):
                ps = pspool.tile([128, SBLK], f32, tag="fc", name="psf2")
                for fp in range(NFF // 2):
                    nc.tensor.matmul(ps,
                                     lhsT=wf2_sb[:, 2 * fp:2 * fp + 2,
                                                 c * 128:(c + 1) * 128],
                                     rhs=ff1_8[:, 2 * fp:2 * fp + 2, :SBLK],
                                     perf_mode=DR,
                                     start=(fp == 0), stop=(fp == NFF // 2 - 1))
                x3c = statpool.tile([128, SBLK], bf16, tag="x3", name="x3")
                nc.vector.tensor_scalar(out=x3c, in0=ps,
                                        scalar1=1.0 / 16, scalar2=f2b_sb[:, c:c + 1],
                                        op0=OP.mult, op1=OP.add)
                for ci, (coff, w) in enumerate(chunks):
                    tr = pspool.tile([128, 128], bf16, tag="fc", name="trf")
                    nc.tensor.transpose(tr[:w, :], x3c[:, coff:coff + w], ident)
                    nc.vector.tensor_tensor(
                        out=o_toks[ci][:w, c * 128:(c + 1) * 128],
                        in0=tr[:w, :],
                        in1=x2_tiles[ci][:w, c * 128:(c + 1) * 128], op=OP.add)
            for ci, (coff, w) in enumerate(chunks):
                nc.sync.dma_start(out=out_d[t0 + coff: t0 + coff + w, :],
                                  in_=o_toks[ci][:w])

        a0 = stage_A_ln(0)
        cur = (stage_A_tr(a0[0]), a0[1])
        next_ln = None
        fc_pending = None
        for isb in range(NSB):
            t0 = isb * SBLK
            hT8, x_tiles = cur

            # ---- stage B: q/k projections (fp8 DoubleRow, weights 16x).
            # qT/kT inherit hT8's 80-padded per-sequence column layout. ----
            qT = [actpool.tile([128, 320], bf16, tag=f"qT{c}", name=f"qT{c}")
                  for c in range(NCH)]
            kT = [actpool.tile([128, 320], bf16, tag=f"kT{c}", name=f"kT{c}")
                  for c in range(NCH)]
            for dst, w_sb, b_sb in ((qT, wq_sb, qb_sb), (kT, wk_sb, kb_sb)):
                for c in range(NCH):
                    ps = pspool.tile([128, 320], f32, tag="big", name="psqkv")
                    for dp in range(NCH // 2):
                        nc.tensor.matmul(ps,
                                         lhsT=w_sb[:, 2 * dp:2 * dp + 2,
                                                   c * 128:(c + 1) * 128],
                                         rhs=hT8[:, 2 * dp:2 * dp + 2, :],
                                         perf_mode=DR,
                                         start=(dp == 0), stop=(dp == NCH // 2 - 1))
                    nc.vector.tensor_scalar(out=dst[c], in0=ps,
                                            scalar1=1.0 / 16,
                                            scalar2=b_sb[:, c:c + 1],
                                            op0=OP.mult, op1=OP.add)

            # ---- stage C: attention per sequence ----
            ctxT = [actpool.tile([128, SBLK], bf16, tag=f"ctxT{c}", name=f"ctxT{c}",
                                 bufs=2)
                    for c in range(NCH)]
            # fp8 feature-major LN2 activations for the DoubleRow FC1
            # (padded to 320 so the k-pair free step is 16B-aligned);
            # double-buffered: stage F consumes it one superblock later.
            h2T8 = actpool.tile([128, NCH, 320], fp8, tag="h2T8", name="h2T8",
                                bufs=2)
            x2_tiles = []
            x2f_tiles = []
            h2toks = []
            next_chunk = 0
            for s in range(G):
                so = s * S        # token-contiguous column base (ctxT)
                sp = s * 80       # 80-padded column base (hT8/qT/kT)
                # v for this sequence, token-major directly (swapped operands,
                # fp8 DoubleRow; the padded hT8 base keeps offsets 16B-aligned)
                vtok = attnpool.tile([S, H, HD], bf16, tag="vtok", name="vtok")
                for half in range(2):
                    psv = pspool.tile([S, 384], f32, tag="big", name="psvtok")
                    for dp in range(NCH // 2):
                        nc.tensor.matmul(psv,
                                         lhsT=hT8[:, 2 * dp:2 * dp + 2,
                                                  sp:sp + S],
                                         rhs=wv_sb[:, 2 * dp:2 * dp + 2,
                                                   half * 384:(half + 1) * 384],
                                         perf_mode=DR,
                                         start=(dp == 0), stop=(dp == NCH // 2 - 1))
                    nc.vector.scalar_tensor_tensor(
                        out=vtok[:, half * 6:(half + 1) * 6, :],
                        in0=psv, scalar=1.0 / 16,
                        in1=vb_bc[:S, half * 384:(half + 1) * 384],
                        op0=OP.mult, op1=OP.add)
                # scoresT[k, q] per head; even heads -> bank A, odd -> bank B.
                # Row-group packing: even heads live on partitions 0:64 of
                # their qT/kT chunk, odd heads on 64:128 -> pairs overlap.
                psc = [pspool.tile([S, SW], f32, tag="sc", name=f"psc{a}")
                       for a in range(2)]
                # the causal mask OPENS each accumulation group (it has no
                # data dependencies, so it fires the moment the PSUM slot
                # frees); the scores accumulate on top and EXP follows the
                # last score matmul directly -- the mask matmul is off the
                # scores->EXP critical chain entirely.
                for a in range(2):
                    nc.tensor.matmul(psc[a], lhsT=ident[:S, :S], rhs=mask6_sb,
                                     start=True, stop=False,
                                     skip_group_check=True)
                for j in range(NH2):
                    for a in range(2):
                        h = 2 * j + a
                        c, po = h // 2, 64 * (h % 2)
                        nc.tensor.matmul(psc[a][:, j * S:(j + 1) * S],
                                         lhsT=kT[c][po:po + 64, sp:sp + S],
                                         rhs=qT[c][po:po + 64, sp:sp + S],
                                         start=False, stop=(j == NH2 - 1),
                                         skip_group_check=True)
                pT = attnpool.tile([S, 2, SW], bf16, tag="pT", name="pT")
                for a in range(2):
                    nc.scalar.activation(out=pT[:, a, :], in_=psc[a], func=AF.Exp)
                # denominators broadcast across partitions: ones.T @ pT, the
                # two banks column-packed into one PSUM tile (concurrent).
                dben = pspool.tile([128, SW], f32, tag="sc", name="dben")
                rp = attnpool.tile([128, SW], bf16, tag="rp", name="rp")
                lnd = attnpool.tile([128, SW], f32, tag="lnd", name="lnd")
                for a in range(2):
                    nc.tensor.matmul(dben[64 * a:64 * a + 64, :],
                                     lhsT=ones77[:, 64 * a:64 * a + 64],
                                     rhs=pT[:, a, :],
                                     start=True, stop=True,
                                     skip_group_check=True)
                # reciprocal as exp(-ln(x)) on the scalar engine: same table
                # set as the attention EXP, and off the busy vector engine
                # (nc.vector.reciprocal is ~6.5ns/elem - 3us per call here).
                # One full-width op per step: ACT cost scales with free dim
                # only, so [128,462] costs the same as [64,462].
                nc.scalar.activation(out=lnd, in_=dben, func=AF.Ln)
                nc.scalar.activation(out=rp, in_=lnd, func=AF.Exp, scale=-1.0)
                # ctx per head pair, column-packed into [128, 77] PSUM already
                # in ctxT layout; normalization fused into the evacuation.
                for j in range(NH2):
                    ctxp = pspool.tile([128, S], f32, tag="tr", name="ctxp",
                                       bufs=2)
                    for a in range(2):
                        h = 2 * j + a
                        nc.tensor.matmul(ctxp[64 * a:64 * a + 64, :],
                                         lhsT=vtok[:, h, :],
                                         rhs=pT[:, a, j * S:(j + 1) * S],
                                         start=True, stop=True,
                                         skip_group_check=True)
                    nc.vector.tensor_tensor(out=ctxT[j][:, so:so + S], in0=ctxp,
                                            in1=rp[:, j * S:(j + 1) * S],
                                            op=OP.mult)
                # prefetch next SB's x-loads + LN1 mid-attention (vector/
                # scalar work only; the PE transposes stay at the boundary)
                if s == 1 and isb + 1 < NSB:
                    next_ln = stage_A_ln(isb + 1)

                # emit O-proj + residual + LN2 for chunks fully covered
                done_tokens = (s + 1) * S
                while (next_chunk < len(chunks)
                       and chunks[next_chunk][0] + chunks[next_chunk][1]
                       <= done_tokens):
                    ci = next_chunk
                    stage_D_chunk(ci, ctxT, x_tiles, x2_tiles, x2f_tiles)
                    coff, w = chunks[ci]
                    h2toks.append(ln_normalize(x2_tiles[ci], w, "E", bufs=3))
                    next_chunk += 1

            # ---- next superblock's LN1 transposes (LN itself already ran
            # mid-attention); clustered here so the PE's transpose-mode
            # switches stay batched ----
            if isb + 1 < NSB:
                cur = (stage_A_tr(next_ln[0]), next_ln[1])

            # E transposes (emitted after C so the in-order PE isn't blocked
            # mid-attention waiting on the LN chains; batched here so the
            # PE's transpose-mode switches stay clustered)
            for ci, (coff, w) in enumerate(chunks):
                ln_transpose(h2toks[ci], coff, w,
                             lambda c, o, ww: h2T8[:, c, o:o + ww], "E")

            # ---- the WHOLE MLP of the previous superblock (FC1 then FC2)
            # emitted here at low priority: FC1(i-1) is ready from the start
            # of this SB, so the filler stream covers this SB's attention
            # stalls, and FC2(i-1) remnants are still in flight to cover
            # the boundary transpose chains.  (FC1's silu<->exp table loads
            # cost ~1.3us each on the scalar engine, but ACT has slack and
            # the PE overlap is worth more.) ----
            if fc_pending is not None:
                pt0, ph2T8, px2 = fc_pending
                ff1_8 = stage_FC1(ph2T8)
                stage_FC2(pt0, ff1_8, px2)
            fc_pending = (t0, h2T8, x2_tiles)
        pt0, ph2T8, px2 = fc_pending
        ff1_8 = stage_FC1(ph2T8)
        stage_FC2(pt0, ff1_8, px2)

    # Restrict the activation-table-set chooser to the two sets that cover
    # everything this kernel uses (ln+exp share one set; silu the other).
    # Entries keep their original indices (act_func_set_id is positional);
    # unwanted sets are just emptied so the chooser can never pick them.
    from concourse.hw_specs import get_activation_tables
    import bass_rust as _bass_rust
    _tables = list(get_activation_tables(nc.m.arch).items())
    _keep = {"natural_log_exp_and_others", "silu_and_others"}
    _tables = [(n, (f if n in _keep else set())) for (n, f) in _tables]

    def _patched_insert_act_table_loads():
        _bass_rust.insert_act_table_loads(nc, _tables)

    nc.insert_act_table_loads = _patched_insert_act_table_loads
    nc.compile()
    return nc


def prep_shared(inputs):
    """Fold LN affine params / scale constants into weights -> shared in_map entries."""
    bf = ml_dtypes.bfloat16
    f32 = np.float32
    g = {k: np.asarray(v, dtype=np.float32) for k, v in inputs.items() if k != "x"}

    # projection/MLP weights in fp8 e4m3, scaled 16x so the smallest weights
    # stay out of the subnormal floor; the 1/16 is folded into the PSUM
    # evacuations (tensor_scalar mult / SILU input scale).
    e4 = ml_dtypes.float8_e4m3
    wqT = (g["ln1_w"][:, None] * g["qw"].T * 0.125 * 16.0).astype(e4)
    wkT = (g["ln1_w"][:, None] * g["kw"].T * 16.0).astype(e4)
    wvT = (g["ln1_w"][:, None] * g["vw"].T * 16.0).astype(e4)
    woT = np.ascontiguousarray(g["ow"].T).astype(bf)
    fc1T = (g["ln2_w"][:, None] * g["fc1_w"].T * 16.0).astype(e4)
    fc2T = (g["fc2_w"].T / 1.702 * 16.0).astype(e4)

    qb = ((g["ln1_b"] @ g["qw"].T + g["qb"]) * 0.125).astype(f32)
    kb = (g["ln1_b"] @ g["kw"].T + g["kb"]).astype(f32)
    vb = (g["ln1_b"] @ g["vw"].T + g["vb"]).astype(f32)
    ob = g["ob"].astype(f32)
    fc1b = ((g["ln2_b"] @ g["fc1_w"].T + g["fc1_b"]) * 1.702).astype(f32)
    fc2b = g["fc2_b"].astype(f32)

    # additive causal mask in scoresT[k, q] orientation (k > q masked),
    # tiled 6x along q for the per-bank [77, 462] accumulation matmul.
    m1 = np.where(np.arange(S)[:, None] > np.arange(S)[None, :], MASK_NEG, 0.0)
    mask6 = np.tile(m1.astype(np.float32), (1, H // 2)).astype(bf)

    return dict(wqT=wqT, wkT=wkT, wvT=wvT, woT=woT, fc1T=fc1T, fc2T=fc2T,
                qb=qb, kb=kb, vb=vb, ob=ob, fc1b=fc1b, fc2b=fc2b, mask6=mask6)


def prep_host_inputs(inputs):
    shared = prep_shared(inputs)
    x = np.asarray(inputs["x"], dtype=np.float32)
    in_maps = []
    for c in range(N_CORES):
        xc = np.ascontiguousarray(
            x[c * BPC:(c + 1) * BPC].reshape(T_CORE, D).astype(np.float32))
        in_maps.append(dict(shared, x=xc))
    return in_maps


_CACHED_NC = None


def _get_nc():
    global _CACHED_NC
    if _CACHED_NC is None:
        _CACHED_NC = build_program()
    return _CACHED_NC


def run(inputs, trace=False):
    from concourse.bass_utils import run_bass_kernel_spmd
    nc = _get_nc()
    in_maps = prep_host_inputs(inputs)
    res = run_bass_kernel_spmd(nc, in_maps, list(range(N_CORES)), trace=trace)
    outs = [np.asarray(res.results[c]["out"], dtype=np.float32).reshape(BPC, S, D)
            for c in range(N_CORES)]
    full = np.concatenate(outs, axis=0)
    return full, res


def kernel(**inputs):
    full, _ = run(inputs, trace=False)
    return full

